# revision 4
# baseline (speedup 1.0000x reference)
"""Channel self-attention module (CSMA) on 8 Trainium2 NeuronCores.

Math: with x [B,C,N,H,W], C==HID==OUT==128, L=N*H*W, the module is
    q = Wq x + bq ; k = Wk x + bk ; v = Wv x + bv          (per-batch [C,L])
    A = softmax(q k^T)                                     ([C,C], rows)
    out = Wo (A v) + bo + x ; result = mean_N(out)         ([C,H*W])

Everything except the softmax is linear in x, so per batch only two small
sufficient statistics of x are needed:
    G = x x^T  [C,C]   and   s = x 1_L  [C]
    logits = Wq G Wk^T + (Wq s) bk^T + bq (Wk s)^T + L bq bk^T
    A = softmax(logits)
    result = (Wo A Wv + I) x_mean + (Wo A bv + bo)
where x_mean = mean over N of x (shape [C, H*W]) and s = 16 * rowsum(x_mean).

Device pass 1 computes G and x_mean in one sweep: x is pre-transposed on the
host to l-major fp16 chunks [128l, 128c]; each chunk is the stationary operand
for (a) a Gram accumulation [C,C] and (b) a block-identity stream that
scatters/accumulates the chunk into the x_mean PSUM window for its hw range.
Pass 2 is a tiny serial tail of [128,128]-scale matmuls + softmax.

Sharding: data-parallel over batch — core b handles batch element b.
"""

import numpy as np

B, C, N, H, W = 8, 128, 16, 56, 56
HW = H * W            # 3136
L = N * HW            # 50176
T = L // 128          # 392 chunks of 128 l-values
TPJ = 49              # chunks per DMA tile
J = T // TPJ          # 8 DMA tiles
N_CORES = 8

_last_results = None  # BassKernelResults of the most recent run (for profiling)


def _apply_tile_drain_patch():
    """This container's walrus build rejects CTRL-format instructions carrying
    more than one semaphore wait, and Tile's end-of-kernel Drain aggregates all
    outstanding waits onto one Drain. Re-emit them as single-wait nops."""
    import concourse.mybir as mybir
    from concourse.tile import TileContext
    from concourse.vector_clock import ScopedClock

    if getattr(TileContext, "_drain_patch_applied", False):
        return

    def _split_drain_and_barrier(self, tick_clock, wait_clock):
        probe = self.nc.sync.nop(nofuse=True)
        wait_clock.add_sem_waits(
            probe.ins, ScopedClock({None: tick_clock.global_clock})
        )
        si = probe.ins.sync_info
        waits = list(si.on_wait) if si is not None else []
        if len(waits) > 1:
            probe.ins.sync_info = mybir.SyncInfo(
                on_wait=waits[:1], on_update=list(si.on_update)
            )
            for w in waits[1:]:
                n = self.nc.sync.nop(nofuse=True)
                n.ins.sync_info = mybir.SyncInfo(on_wait=[w], on_update=[])
        self.nc.sync.drain()
        self.nc.all_engine_barrier()
        assert self.sems is not None
        popped = self.nc._tile_sem_poison_stack.pop()
        assert popped is self._sem_poison
        self.nc.clear_and_free_semaphores(list(self.sems.allocated().values()))
        self.nc.all_engine_barrier()

    TileContext._drain_and_barrier = _split_drain_and_barrier
    TileContext._drain_patch_applied = True


def _split_multi_waits(nc, max_waits=1):
    """This walrus build supports at most one semaphore wait per instruction.
    Move surplus waits onto single-wait nops inserted just before the
    instruction on the same engine (the sequencer blocks on them in order, so
    the guarded instruction still issues only after every wait clears)."""
    import concourse.mybir as mybir

    k = 0
    for f in nc.m.functions:
        for b in f.blocks:
            il = list(b.instructions)
            new = []
            changed = False
            for inst in il:
                si = inst.sync_info
                waits = list(si.on_wait) if si is not None else []
                if len(waits) > max_waits:
                    changed = True
                    for w in waits[:-max_waits]:
                        nop = mybir.InstNoOp(name=f"Wsplit-{k}", ins=[], outs=[])
                        k += 1
                        nop.engine = inst.engine
                        nop.sync_info = mybir.SyncInfo(on_wait=[w], on_update=[])
                        new.append(nop)
                    inst.sync_info = mybir.SyncInfo(
                        on_wait=waits[-max_waits:], on_update=list(si.on_update)
                    )
                new.append(inst)
            if changed:
                b.instructions = new


def _window_pieces(w0):
    """Split the hw window [w0, w0+128) into pieces that neither wrap 3136 nor
    cross a 512-wide PSUM bank boundary. Returns (dst_hw, src_col, width)."""
    if w0 + 128 <= HW:
        segs = [(w0, 0, 128)]
    else:
        r = HW - w0
        segs = [(w0, 0, r), (0, r, 128 - r)]
    out = []
    for d, s, n in segs:
        while n > 0:
            m = min(n, 512 - (d % 512))
            out.append((d, s, m))
            d += m
            s += m
            n -= m
    return out


def _build_nc():
    import concourse.bass as bass
    import concourse.mybir as mybir
    from concourse.tile import TileContext

    _apply_tile_drain_patch()

    f32, f16 = mybir.dt.float32, mybir.dt.float16
    nc = bass.Bass()

    xt = nc.dram_tensor("xt", [128, L], f16, kind="ExternalInput")
    wqT_d = nc.dram_tensor("wqT", [128, 128], f32, kind="ExternalInput")
    wkT_d = nc.dram_tensor("wkT", [128, 128], f32, kind="ExternalInput")
    wv_d = nc.dram_tensor("wv", [128, 128], f32, kind="ExternalInput")
    woT_d = nc.dram_tensor("woT", [128, 128], f32, kind="ExternalInput")
    bq_d = nc.dram_tensor("bq_row", [1, 128], f32, kind="ExternalInput")
    bk_d = nc.dram_tensor("bk_row", [1, 128], f32, kind="ExternalInput")
    bv_d = nc.dram_tensor("bv_col", [128, 1], f32, kind="ExternalInput")
    bo_d = nc.dram_tensor("bo_col", [128, 1], f32, kind="ExternalInput")
    ic_d = nc.dram_tensor("ic16", [128, 128], f16, kind="ExternalInput")
    id_d = nc.dram_tensor("ident", [128, 128], f32, kind="ExternalInput")
    out_d = nc.dram_tensor("out", [128, HW], f32, kind="ExternalOutput")

    # Per-bank first/last writer schedule for the x_mean PSUM banks (7 banks:
    # 6x512 + 1x64). Bank index of hw offset d is d // 512.
    writes_per_bank = [0] * 7
    for t in range(T):
        for d, s, n in _window_pieces((128 * t) % HW):
            writes_per_bank[d // 512] += 1

    with TileContext(nc) as tc:
        with (
            tc.tile_pool(name="consts", bufs=1) as consts,
            tc.tile_pool(name="xtiles", bufs=2) as xtiles,
            tc.tile_pool(name="sbres", bufs=1) as sbres,
        ):
            # constants
            ic_sb = consts.tile([128, 128], f16)
            nc.sync.dma_start(out=ic_sb[:], in_=ic_d[:])
            wqT_sb = consts.tile([128, 128], f32)
            nc.sync.dma_start(out=wqT_sb[:], in_=wqT_d[:])
            wkT_sb = consts.tile([128, 128], f32)
            nc.sync.dma_start(out=wkT_sb[:], in_=wkT_d[:])
            wv_sb = consts.tile([128, 128], f32)
            nc.sync.dma_start(out=wv_sb[:], in_=wv_d[:])
            woT_sb = consts.tile([128, 128], f32)
            nc.sync.dma_start(out=woT_sb[:], in_=woT_d[:])
            id_sb = consts.tile([128, 128], f32)
            nc.sync.dma_start(out=id_sb[:], in_=id_d[:])
            bq_sb = consts.tile([1, 128], f32)
            nc.sync.dma_start(out=bq_sb[:], in_=bq_d[:])
            bk_sb = consts.tile([1, 128], f32)
            nc.sync.dma_start(out=bk_sb[:], in_=bk_d[:])
            bv_sb = consts.tile([128, 1], f32)
            nc.sync.dma_start(out=bv_sb[:], in_=bv_d[:])
            bo_sb = consts.tile([128, 1], f32)
            nc.sync.dma_start(out=bo_sb[:], in_=bo_d[:])

            # ---- pass 1: G = x x^T and x_mean, one sweep over x^T chunks ----
            with tc.tile_pool(name="ps1", bufs=1, space="PSUM") as ps1:
                g_ps = ps1.tile([128, 128], f32)
                xm_ps = [
                    ps1.tile([128, 512], f32, name=f"xm{k}", tag=f"xm{k}")
                    for k in range(6)
                ]
                xm_ps.append(ps1.tile([128, 64], f32, name="xm6", tag="xm6"))

                seen_per_bank = [0] * 7
                for j in range(J):
                    xt_sb = xtiles.tile([128, TPJ * 128], f16)
                    nc.sync.dma_start(
                        out=xt_sb[:], in_=xt[:, 6272 * j : 6272 * (j + 1)]
                    )
                    for i in range(TPJ):
                        t = TPJ * j + i
                        ch = xt_sb[:, 128 * i : 128 * (i + 1)]
                        nc.tensor.matmul(
                            g_ps[:],
                            lhsT=ch,
                            rhs=ch,
                            start=(t == 0),
                            stop=(t == T - 1),
                        )
                        for d, s, n in _window_pieces((128 * t) % HW):
                            bk_i = d // 512
                            seen_per_bank[bk_i] += 1
                            nc.tensor.matmul(
                                xm_ps[bk_i][:, d % 512 : d % 512 + n],
                                lhsT=ch,
                                rhs=ic_sb[:, s : s + n],
                                start=(seen_per_bank[bk_i] == 1),
                                stop=(seen_per_bank[bk_i] == writes_per_bank[bk_i]),
                            )

                g_sb = sbres.tile([128, 128], f32)
                nc.vector.tensor_copy(out=g_sb[:], in_=g_ps[:])
                xm_sb = sbres.tile([128, HW], f32)
                for k in range(7):
                    wdt = 64 if k == 6 else 512
                    nc.scalar.copy(
                        out=xm_sb[:, 512 * k : 512 * k + wdt], in_=xm_ps[k][:]
                    )

            # ---- pass 2: tiny serial tail ----
            with tc.tile_pool(name="ps2", bufs=1, space="PSUM") as ps2:
                # s = 16 * rowsum(x_mean)
                s_raw = sbres.tile([128, 1], f32)
                nc.vector.reduce_sum(
                    out=s_raw[:], in_=xm_sb[:], axis=mybir.AxisListType.X
                )
                s_col = sbres.tile([128, 1], f32)
                nc.scalar.mul(out=s_col[:], in_=s_raw[:], mul=16.0)

                # s^T and (Wk s)^T as 1-partition rows
                rows_ps = ps2.tile([1, 512], f32, tag="rows")
                nc.tensor.matmul(
                    rows_ps[:, 0:128], lhsT=s_col[:], rhs=id_sb[:],
                    start=True, stop=True,
                )
                nc.tensor.matmul(
                    rows_ps[:, 128:256], lhsT=s_col[:], rhs=wkT_sb[:],
                    start=True, stop=True, skip_group_check=True,
                )
                srow_sb = sbres.tile([1, 128], f32)
                nc.scalar.copy(out=srow_sb[:], in_=rows_ps[:, 0:128])
                kkrow_sb = sbres.tile([1, 128], f32)
                nc.scalar.copy(out=kkrow_sb[:], in_=rows_ps[:, 128:256])

                # r2 = L*bk + (Wk s)^T   [1,128]
                r2_sb = sbres.tile([1, 128], f32)
                nc.vector.scalar_tensor_tensor(
                    out=r2_sb[:],
                    in0=bk_sb[:],
                    scalar=float(L),
                    in1=kkrow_sb[:],
                    op0=mybir.AluOpType.mult,
                    op1=mybir.AluOpType.add,
                )

                # V1 = G Wk^T + s bk^T
                v1_ps = ps2.tile([128, 128], f32, tag="mm")
                nc.tensor.matmul(
                    v1_ps[:], lhsT=g_sb[:], rhs=wkT_sb[:], start=True, stop=False
                )
                nc.tensor.matmul(
                    v1_ps[:], lhsT=srow_sb[:], rhs=bk_sb[:], start=False, stop=True
                )
                v1_sb = sbres.tile([128, 128], f32)
                nc.vector.tensor_copy(out=v1_sb[:], in_=v1_ps[:])

                # logits = Wq V1 + bq r2^T... (bq outer r2)
                lg_ps = ps2.tile([128, 128], f32, tag="mm2")
                nc.tensor.matmul(
                    lg_ps[:], lhsT=wqT_sb[:], rhs=v1_sb[:], start=True, stop=False
                )
                nc.tensor.matmul(
                    lg_ps[:], lhsT=bq_sb[:], rhs=r2_sb[:], start=False, stop=True
                )

                # softmax over free axis
                negmax = sbres.tile([128, 1], f32)
                nc.vector.tensor_reduce(
                    out=negmax[:], in_=lg_ps[:], axis=mybir.AxisListType.X,
                    op=mybir.AluOpType.max, negate=True,
                )
                a_sb = sbres.tile([128, 128], f32)
                sumexp = sbres.tile([128, 1], f32)
                nc.scalar.activation(
                    out=a_sb[:], in_=lg_ps[:],
                    func=mybir.ActivationFunctionType.Exp,
                    bias=negmax[:], scale=1.0, accum_out=sumexp[:],
                )
                rec = sbres.tile([128, 1], f32)
                nc.vector.reciprocal(out=rec[:], in_=sumexp[:])
                nc.vector.tensor_scalar_mul(a_sb[:], a_sb[:], rec[:])

                # U = A^T Wo^T  [b, o]
                u_ps = ps2.tile([128, 128], f32, tag="mm")
                nc.tensor.matmul(
                    u_ps[:], lhsT=a_sb[:], rhs=woT_sb[:], start=True, stop=True
                )
                u_sb = sbres.tile([128, 128], f32)
                nc.vector.tensor_copy(out=u_sb[:], in_=u_ps[:])

                # M^T = Wv^T A^T Wo^T ; P^T = M^T + I
                mt_ps = ps2.tile([128, 128], f32, tag="mm2")
                nc.tensor.matmul(
                    mt_ps[:], lhsT=wv_sb[:], rhs=u_sb[:], start=True, stop=True
                )
                pt_sb = sbres.tile([128, 128], f32)
                nc.vector.tensor_add(out=pt_sb[:], in0=mt_ps[:], in1=id_sb[:])

                # cvec = U^T bv + bo  [o,1]
                cv_ps = ps2.tile([128, 64], f32, tag="cv")
                nc.tensor.matmul(
                    cv_ps[:, 0:1], lhsT=u_sb[:], rhs=bv_sb[:], start=True, stop=True
                )
                cvec_sb = sbres.tile([128, 1], f32)
                nc.scalar.activation(
                    out=cvec_sb[:], in_=cv_ps[:, 0:1],
                    func=mybir.ActivationFunctionType.Identity,
                    bias=bo_sb[:], scale=1.0,
                )

                # out = P^T^T x_mean + cvec  (7 chunks of 448 columns)
                out_sb = sbres.tile([128, HW], f32)
                for k in range(7):
                    oc_ps = ps2.tile([128, 448], f32, tag="oc")
                    nc.tensor.matmul(
                        oc_ps[:],
                        lhsT=pt_sb[:],
                        rhs=xm_sb[:, 448 * k : 448 * (k + 1)],
                        start=True, stop=True,
                    )
                    nc.scalar.activation(
                        out=out_sb[:, 448 * k : 448 * (k + 1)], in_=oc_ps[:],
                        func=mybir.ActivationFunctionType.Identity,
                        bias=cvec_sb[:], scale=1.0,
                    )
                nc.sync.dma_start(out=out_d[:], in_=out_sb[:])

    _split_multi_waits(nc)
    return nc


_cached_nc = None


def kernel(x, w_q, b_q, w_k, b_k, w_v, b_v, w_o, b_o):
    global _cached_nc, _last_results
    from concourse.bass_utils import run_bass_kernel_spmd

    if _cached_nc is None:
        _cached_nc = _build_nc()
    nc = _cached_nc

    x = np.asarray(x, np.float32)
    consts = {
        "wqT": np.ascontiguousarray(np.asarray(w_q, np.float32).T),
        "wkT": np.ascontiguousarray(np.asarray(w_k, np.float32).T),
        "wv": np.ascontiguousarray(np.asarray(w_v, np.float32)),
        "woT": np.ascontiguousarray(np.asarray(w_o, np.float32).T),
        "bq_row": np.asarray(b_q, np.float32).reshape(1, 128),
        "bk_row": np.asarray(b_k, np.float32).reshape(1, 128),
        "bv_col": np.ascontiguousarray(np.asarray(b_v, np.float32).reshape(128, 1)),
        "bo_col": np.ascontiguousarray(np.asarray(b_o, np.float32).reshape(128, 1)),
        "ic16": np.ascontiguousarray((np.eye(128) / 16.0).astype(np.float16)),
        "ident": np.ascontiguousarray(np.eye(128, dtype=np.float32)),
    }
    in_maps = []
    for b in range(B):
        # xt[p, 128*t + c] = x[b, c, 128*t + p]  (l-major chunks, fp16)
        xb = x[b].reshape(C, T, 128)
        xt_b = np.ascontiguousarray(
            xb.transpose(2, 1, 0).reshape(128, L).astype(np.float16)
        )
        in_maps.append({"xt": xt_b, **consts})

    res = run_bass_kernel_spmd(nc, in_maps, list(range(N_CORES)))
    _last_results = res

    out = np.empty((B, C, H, W), np.float32)
    for b in range(B):
        out[b] = res.results[b]["out"].reshape(C, H, W)
    return out


# revision 5
# speedup vs baseline: 1.1882x; 1.1882x over previous
"""Channel self-attention module (CSMA) on 8 Trainium2 NeuronCores.

Math: with x [B,C,N,H,W], C==HID==OUT==128, L=N*H*W, the module is
    q = Wq x + bq ; k = Wk x + bk ; v = Wv x + bv          (per-batch [C,L])
    A = softmax(q k^T)                                     ([C,C], rows)
    out = Wo (A v) + bo + x ; result = mean_N(out)         ([C,H*W])

Everything except the softmax is linear in x, so per batch only two small
sufficient statistics of x are needed:
    G = x x^T  [C,C]   and   s = x 1_L  [C]
    logits = Wq G Wk^T + (Wq s) bk^T + bq (Wk s)^T + L bq bk^T
    A = softmax(logits)
    result = (Wo A Wv + I) x_mean + (Wo A bv + bo)
where x_mean = mean over N of x (shape [C, H*W]) and s = 16 * rowsum(x_mean).

Device pass 1 computes G and x_mean in one sweep: x is pre-transposed on the
host to l-major fp16 chunks [128l, 128c]; each chunk is the stationary operand
for (a) a Gram accumulation [C,C] and (b) a (1/16)-identity stream that
accumulates the chunk into the x_mean PSUM window for its hw range.
Pass 2 is a tiny serial tail of [128,128]-scale matmuls + softmax.

Sharding: data-parallel over batch — core b handles batch element b.
"""

import numpy as np

B, C, N, H, W = 8, 128, 16, 56, 56
HW = H * W            # 3136
L = N * HW            # 50176
T = L // 128          # 392 chunks of 128 l-values
TPJ = 28              # chunks per DMA tile
J = T // TPJ          # 14 DMA tiles
N_CORES = 8

# fp32 const-pack column layout
_WQ, _WK, _WV, _WO, _ID = 0, 128, 256, 384, 512
_BV, _BO, _BQ, _BK = 640, 641, 642, 770
_PACKW = 898

_last_results = None  # BassKernelResults of the most recent run (for profiling)


def _apply_env_patches():
    """Two workarounds for this container's walrus build plus one perf flag.

    1. Tile's end-of-kernel Drain aggregates every outstanding sem wait onto
       one CTRL instruction, but this walrus rejects >1 wait per instruction
       ("Too many sync wait commands"): re-emit surplus waits as single-wait
       nops (see _split_multi_waits, applied post-build).
    2. --enable-ldw-opt=true: consecutive matmuls sharing a stationary
       operand skip the redundant LDWEIGHTS reload (our Gram + x_mean matmul
       pairs share their chunk).
    """
    import concourse.mybir as mybir
    import concourse.bass_utils as bu
    from concourse.tile import TileContext
    from concourse.vector_clock import ScopedClock

    if not getattr(TileContext, "_drain_patch_applied", False):

        def _split_drain_and_barrier(self, tick_clock, wait_clock):
            probe = self.nc.sync.nop(nofuse=True)
            wait_clock.add_sem_waits(
                probe.ins, ScopedClock({None: tick_clock.global_clock})
            )
            si = probe.ins.sync_info
            waits = list(si.on_wait) if si is not None else []
            if len(waits) > 1:
                probe.ins.sync_info = mybir.SyncInfo(
                    on_wait=waits[:1], on_update=list(si.on_update)
                )
                for w in waits[1:]:
                    n = self.nc.sync.nop(nofuse=True)
                    n.ins.sync_info = mybir.SyncInfo(on_wait=[w], on_update=[])
            self.nc.sync.drain()
            self.nc.all_engine_barrier()
            assert self.sems is not None
            popped = self.nc._tile_sem_poison_stack.pop()
            assert popped is self._sem_poison
            self.nc.clear_and_free_semaphores(list(self.sems.allocated().values()))
            self.nc.all_engine_barrier()

        TileContext._drain_and_barrier = _split_drain_and_barrier
        TileContext._drain_patch_applied = True

    if not getattr(bu, "_ldw_opt_patch_applied", False):
        orig = bu.get_walrus_args

        def _walrus_args_ldw_opt(*a, **kw):
            return [
                arg.replace("--enable-ldw-opt=false", "--enable-ldw-opt=true")
                for arg in orig(*a, **kw)
            ]

        bu.get_walrus_args = _walrus_args_ldw_opt
        bu._ldw_opt_patch_applied = True


def _split_multi_waits(nc, max_waits=1):
    """Move surplus semaphore waits onto single-wait nops inserted just before
    the owning instruction on the same engine (the sequencer executes them in
    order, so the guarded instruction still issues only after all waits)."""
    import concourse.mybir as mybir

    k = 0
    for f in nc.m.functions:
        for b in f.blocks:
            il = list(b.instructions)
            new = []
            changed = False
            for inst in il:
                si = inst.sync_info
                waits = list(si.on_wait) if si is not None else []
                if len(waits) > max_waits:
                    changed = True
                    for w in waits[:-max_waits]:
                        nop = mybir.InstNoOp(name=f"Wsplit-{k}", ins=[], outs=[])
                        k += 1
                        nop.engine = inst.engine
                        nop.sync_info = mybir.SyncInfo(on_wait=[w], on_update=[])
                        new.append(nop)
                    inst.sync_info = mybir.SyncInfo(
                        on_wait=waits[-max_waits:], on_update=list(si.on_update)
                    )
                new.append(inst)
            if changed:
                b.instructions = new


def _window_pieces(w0):
    """Split the hw window [w0, w0+128) into pieces that neither wrap 3136 nor
    cross a 512-wide PSUM bank boundary. Returns (dst_hw, src_col, width)."""
    if w0 + 128 <= HW:
        segs = [(w0, 0, 128)]
    else:
        r = HW - w0
        segs = [(w0, 0, r), (0, r, 128 - r)]
    out = []
    for d, s, n in segs:
        while n > 0:
            m = min(n, 512 - (d % 512))
            out.append((d, s, m))
            d += m
            s += m
            n -= m
    return out


def _build_nc():
    import concourse.bass as bass
    import concourse.mybir as mybir
    from concourse.tile import TileContext

    _apply_env_patches()

    f32, f16 = mybir.dt.float32, mybir.dt.float16
    nc = bass.Bass()

    xt = nc.dram_tensor("xt", [128, L], f16, kind="ExternalInput")
    ic_d = nc.dram_tensor("ic16", [128, 128], f16, kind="ExternalInput")
    pk_d = nc.dram_tensor("pack", [128, _PACKW], f32, kind="ExternalInput")
    out_d = nc.dram_tensor("out", [128, HW], f32, kind="ExternalOutput")

    writes_per_bank = [0] * 7
    for t in range(T):
        for d, s, n in _window_pieces((128 * t) % HW):
            writes_per_bank[d // 512] += 1

    with TileContext(nc) as tc:
        with (
            tc.tile_pool(name="consts", bufs=1) as consts,
            tc.tile_pool(name="xtiles", bufs=3) as xtiles,
            tc.tile_pool(name="sbres", bufs=1) as sbres,
        ):
            # first x tile + the two const loads go out immediately; x tiles
            # alternate between the two HWDGE queues (sync / scalar).
            xt_sbs = []
            xt_sb0 = xtiles.tile([128, TPJ * 128], f16, name="xt_sb0", tag="xt")
            nc.sync.dma_start(out=xt_sb0[:], in_=xt[:, 0 : TPJ * 128])
            xt_sbs.append(xt_sb0)
            ic_sb = consts.tile([128, 128], f16)
            nc.scalar.dma_start(out=ic_sb[:], in_=ic_d[:])
            pk_sb = consts.tile([128, _PACKW], f32)
            nc.scalar.dma_start(out=pk_sb[:], in_=pk_d[:])

            wqT_sb = pk_sb[:, _WQ : _WQ + 128]
            wkT_sb = pk_sb[:, _WK : _WK + 128]
            wv_sb = pk_sb[:, _WV : _WV + 128]
            woT_sb = pk_sb[:, _WO : _WO + 128]
            id_sb = pk_sb[:, _ID : _ID + 128]
            bv_sb = pk_sb[:, _BV : _BV + 1]
            bo_sb = pk_sb[:, _BO : _BO + 1]
            bq_sb = pk_sb[0:1, _BQ : _BQ + 128]
            bk_sb = pk_sb[0:1, _BK : _BK + 128]

            # ---- pass 1: G = x x^T and x_mean, one sweep over x^T chunks ----
            with tc.tile_pool(name="ps1", bufs=1, space="PSUM") as ps1:
                g_ps = ps1.tile([128, 128], f32)
                xm_ps = [
                    ps1.tile([128, 512], f32, name=f"xm{k}", tag=f"xm{k}")
                    for k in range(6)
                ]
                xm_ps.append(ps1.tile([128, 64], f32, name="xm6", tag="xm6"))

                seen_per_bank = [0] * 7
                for j in range(J):
                    if j == 0:
                        xt_sb = xt_sbs[0]
                    else:
                        xt_sb = xtiles.tile(
                            [128, TPJ * 128], f16, name=f"xt_sb{j}", tag="xt"
                        )
                        eng = nc.sync if j % 2 == 0 else nc.scalar
                        eng.dma_start(
                            out=xt_sb[:],
                            in_=xt[:, TPJ * 128 * j : TPJ * 128 * (j + 1)],
                        )
                    for i in range(TPJ):
                        t = TPJ * j + i
                        ch = xt_sb[:, 128 * i : 128 * (i + 1)]
                        nc.tensor.matmul(
                            g_ps[:],
                            lhsT=ch,
                            rhs=ch,
                            start=(t == 0),
                            stop=(t == T - 1),
                        )
                        for d, s, n in _window_pieces((128 * t) % HW):
                            bk_i = d // 512
                            seen_per_bank[bk_i] += 1
                            nc.tensor.matmul(
                                xm_ps[bk_i][:, d % 512 : d % 512 + n],
                                lhsT=ch,
                                rhs=ic_sb[:, s : s + n],
                                start=(seen_per_bank[bk_i] == 1),
                                stop=(seen_per_bank[bk_i] == writes_per_bank[bk_i]),
                            )

                g_sb = sbres.tile([128, 128], f32)
                nc.vector.tensor_copy(out=g_sb[:], in_=g_ps[:])
                xm_sb = sbres.tile([128, HW], f32)
                for k in range(7):
                    wdt = 64 if k == 6 else 512
                    nc.scalar.copy(
                        out=xm_sb[:, 512 * k : 512 * k + wdt], in_=xm_ps[k][:]
                    )

            # ---- pass 2: tiny serial tail ----
            with tc.tile_pool(name="ps2", bufs=1, space="PSUM") as ps2:
                # s = 16 * rowsum(x_mean)
                s_raw = sbres.tile([128, 1], f32)
                nc.vector.reduce_sum(
                    out=s_raw[:], in_=xm_sb[:], axis=mybir.AxisListType.X
                )
                s_col = sbres.tile([128, 1], f32)
                nc.scalar.mul(out=s_col[:], in_=s_raw[:], mul=16.0)

                # s^T and (Wk s)^T as 1-partition rows
                rows_ps = ps2.tile([1, 512], f32, tag="rows")
                nc.tensor.matmul(
                    rows_ps[:, 0:128], lhsT=s_col[:], rhs=id_sb,
                    start=True, stop=True,
                )
                nc.tensor.matmul(
                    rows_ps[:, 128:256], lhsT=s_col[:], rhs=wkT_sb,
                    start=True, stop=True, skip_group_check=True,
                )
                srow_sb = sbres.tile([1, 128], f32)
                nc.scalar.copy(out=srow_sb[:], in_=rows_ps[:, 0:128])
                kkrow_sb = sbres.tile([1, 128], f32)
                nc.scalar.copy(out=kkrow_sb[:], in_=rows_ps[:, 128:256])

                # r2 = L*bk + (Wk s)^T   [1,128]
                r2_sb = sbres.tile([1, 128], f32)
                nc.vector.scalar_tensor_tensor(
                    out=r2_sb[:],
                    in0=bk_sb,
                    scalar=float(L),
                    in1=kkrow_sb[:],
                    op0=mybir.AluOpType.mult,
                    op1=mybir.AluOpType.add,
                )

                # V1 = G Wk^T + s bk^T
                v1_ps = ps2.tile([128, 128], f32, tag="mm")
                nc.tensor.matmul(
                    v1_ps[:], lhsT=g_sb[:], rhs=wkT_sb, start=True, stop=False
                )
                nc.tensor.matmul(
                    v1_ps[:], lhsT=srow_sb[:], rhs=bk_sb, start=False, stop=True
                )
                v1_sb = sbres.tile([128, 128], f32)
                nc.vector.tensor_copy(out=v1_sb[:], in_=v1_ps[:])

                # logits = Wq V1 + bq (outer) r2
                lg_ps = ps2.tile([128, 128], f32, tag="mm2")
                nc.tensor.matmul(
                    lg_ps[:], lhsT=wqT_sb, rhs=v1_sb[:], start=True, stop=False
                )
                nc.tensor.matmul(
                    lg_ps[:], lhsT=bq_sb, rhs=r2_sb[:], start=False, stop=True
                )

                # softmax over the free axis
                negmax = sbres.tile([128, 1], f32)
                nc.vector.tensor_reduce(
                    out=negmax[:], in_=lg_ps[:], axis=mybir.AxisListType.X,
                    op=mybir.AluOpType.max, negate=True,
                )
                a_sb = sbres.tile([128, 128], f32)
                sumexp = sbres.tile([128, 1], f32)
                nc.scalar.activation(
                    out=a_sb[:], in_=lg_ps[:],
                    func=mybir.ActivationFunctionType.Exp,
                    bias=negmax[:], scale=1.0, accum_out=sumexp[:],
                )
                rec = sbres.tile([128, 1], f32)
                nc.vector.reciprocal(out=rec[:], in_=sumexp[:])
                nc.vector.tensor_scalar_mul(a_sb[:], a_sb[:], rec[:])

                # U = A^T Wo^T  [b, o]
                u_ps = ps2.tile([128, 128], f32, tag="mm")
                nc.tensor.matmul(
                    u_ps[:], lhsT=a_sb[:], rhs=woT_sb, start=True, stop=True
                )
                u_sb = sbres.tile([128, 128], f32)
                nc.vector.tensor_copy(out=u_sb[:], in_=u_ps[:])

                # M^T = Wv^T A^T Wo^T ; P^T = M^T + I
                mt_ps = ps2.tile([128, 128], f32, tag="mm2")
                nc.tensor.matmul(
                    mt_ps[:], lhsT=wv_sb, rhs=u_sb[:], start=True, stop=True
                )
                pt_sb = sbres.tile([128, 128], f32)
                nc.vector.tensor_add(out=pt_sb[:], in0=mt_ps[:], in1=id_sb)

                # cvec = U^T bv + bo  [o,1]
                cv_ps = ps2.tile([128, 64], f32, tag="cv")
                nc.tensor.matmul(
                    cv_ps[:, 0:1], lhsT=u_sb[:], rhs=bv_sb, start=True, stop=True
                )
                cvec_sb = sbres.tile([128, 1], f32)
                nc.scalar.activation(
                    out=cvec_sb[:], in_=cv_ps[:, 0:1],
                    func=mybir.ActivationFunctionType.Identity,
                    bias=bo_sb, scale=1.0,
                )

                # out = (M^T + I)^T x_mean + cvec, 7 chunks of 448 columns,
                # each DMA'd out as soon as its bias-add lands.
                out_sb = sbres.tile([128, HW], f32)
                for k in range(7):
                    oc_ps = ps2.tile([128, 448], f32, name=f"oc{k}", tag="oc")
                    nc.tensor.matmul(
                        oc_ps[:],
                        lhsT=pt_sb[:],
                        rhs=xm_sb[:, 448 * k : 448 * (k + 1)],
                        start=True, stop=True,
                    )
                    nc.scalar.activation(
                        out=out_sb[:, 448 * k : 448 * (k + 1)], in_=oc_ps[:],
                        func=mybir.ActivationFunctionType.Identity,
                        bias=cvec_sb[:], scale=1.0,
                    )
                    eng = nc.sync if k % 2 == 0 else nc.scalar
                    eng.dma_start(
                        out=out_d[:, 448 * k : 448 * (k + 1)],
                        in_=out_sb[:, 448 * k : 448 * (k + 1)],
                    )

    _split_multi_waits(nc)
    return nc


_cached_nc = None


def kernel(x, w_q, b_q, w_k, b_k, w_v, b_v, w_o, b_o):
    global _cached_nc, _last_results
    from concourse.bass_utils import run_bass_kernel_spmd

    if _cached_nc is None:
        _cached_nc = _build_nc()
    nc = _cached_nc

    x = np.asarray(x, np.float32)
    pack = np.zeros((128, _PACKW), np.float32)
    pack[:, _WQ : _WQ + 128] = np.asarray(w_q, np.float32).T
    pack[:, _WK : _WK + 128] = np.asarray(w_k, np.float32).T
    pack[:, _WV : _WV + 128] = np.asarray(w_v, np.float32)
    pack[:, _WO : _WO + 128] = np.asarray(w_o, np.float32).T
    pack[:, _ID : _ID + 128] = np.eye(128, dtype=np.float32)
    pack[:, _BV] = np.asarray(b_v, np.float32)
    pack[:, _BO] = np.asarray(b_o, np.float32)
    pack[0, _BQ : _BQ + 128] = np.asarray(b_q, np.float32)
    pack[0, _BK : _BK + 128] = np.asarray(b_k, np.float32)
    ic16 = np.ascontiguousarray((np.eye(128) / 16.0).astype(np.float16))

    in_maps = []
    for b in range(B):
        # xt[p, 128*t + c] = x[b, c, 128*t + p]  (l-major chunks, fp16)
        xb = x[b].reshape(C, T, 128)
        xt_b = np.ascontiguousarray(
            xb.transpose(2, 1, 0).reshape(128, L).astype(np.float16)
        )
        in_maps.append({"xt": xt_b, "ic16": ic16, "pack": pack})

    res = run_bass_kernel_spmd(nc, in_maps, list(range(N_CORES)))
    _last_results = res

    out = np.empty((B, C, H, W), np.float32)
    for b in range(B):
        out[b] = res.results[b]["out"].reshape(C, H, W)
    return out


# revision 7
# speedup vs baseline: 1.3605x; 1.1450x over previous
"""Channel self-attention module (CSMA) on 8 Trainium2 NeuronCores.

Math: with x [B,C,N,H,W], C==HID==OUT==128, L=N*H*W, the module is
    q = Wq x + bq ; k = Wk x + bk ; v = Wv x + bv          (per-batch [C,L])
    A = softmax(q k^T)                                     ([C,C], rows)
    out = Wo (A v) + bo + x ; result = mean_N(out)         ([C,H*W])

Everything except the softmax is linear in x, so per batch only two small
sufficient statistics of x are needed:
    G = x x^T  [C,C]   and   s = x 1_L  [C]
    logits = Wq G Wk^T + (Wq s) bk^T + bq (Wk s)^T + L bq bk^T
    A = softmax(logits)
    result = (Wo A Wv + I) x_mean + (Wo A bv + bo)
where x_mean = mean over N of x (shape [C, H*W]).

Device pass 1 computes G, s and x_mean in one sweep: x is pre-transposed on
the host to l-major fp16 chunks [128l, 128c] with a ones column appended per
chunk; each chunk is the stationary operand for (a) a [chunk|ones] stream
accumulating [G|s] and (b) a (1/16)-identity stream accumulating the chunk
into the x_mean PSUM window for its hw range. Pass 2 is a short serial tail
of [128,128]-scale fp16 matmuls + softmax, with PSUM drains on the otherwise
idle vector engine.

Sharding: data-parallel over batch — core b handles batch element b.
"""

import numpy as np

B, C, N, H, W = 8, 128, 16, 56, 56
HW = H * W            # 3136
L = N * HW            # 50176
T = L // 128          # 392 chunks of 128 l-values
TPJ = 28              # chunks per DMA tile
J = T // TPJ          # 14 DMA tiles
CW = 129              # chunk width in the xt layout (128 cols + ones column)
N_CORES = 8

# fp16 const-pack column layout
_WQ, _WK, _WV, _WO, _ID = 0, 128, 256, 384, 512
_BV, _BO, _BQ, _BK = 640, 641, 642, 770
_PACKW = 898

_last_results = None  # BassKernelResults of the most recent run (for profiling)


def _apply_env_patches():
    """Workarounds for this container's walrus build.

    1. Tile's end-of-kernel Drain aggregates every outstanding sem wait onto
       one CTRL instruction, but this walrus rejects >1 wait per instruction
       ("Too many sync wait commands"): re-emit surplus waits as single-wait
       nops (see _split_multi_waits, applied post-build).
    2. --enable-ldw-opt=true lets codegen skip redundant LDWEIGHTS reloads
       for consecutive matmuls sharing a stationary operand.
    """
    import concourse.mybir as mybir
    import concourse.bass_utils as bu
    from concourse.tile import TileContext
    from concourse.vector_clock import ScopedClock

    if not getattr(TileContext, "_drain_patch_applied", False):

        def _split_drain_and_barrier(self, tick_clock, wait_clock):
            probe = self.nc.sync.nop(nofuse=True)
            wait_clock.add_sem_waits(
                probe.ins, ScopedClock({None: tick_clock.global_clock})
            )
            si = probe.ins.sync_info
            waits = list(si.on_wait) if si is not None else []
            if len(waits) > 1:
                probe.ins.sync_info = mybir.SyncInfo(
                    on_wait=waits[:1], on_update=list(si.on_update)
                )
                for w in waits[1:]:
                    n = self.nc.sync.nop(nofuse=True)
                    n.ins.sync_info = mybir.SyncInfo(on_wait=[w], on_update=[])
            self.nc.sync.drain()
            self.nc.all_engine_barrier()
            assert self.sems is not None
            popped = self.nc._tile_sem_poison_stack.pop()
            assert popped is self._sem_poison
            self.nc.clear_and_free_semaphores(list(self.sems.allocated().values()))
            self.nc.all_engine_barrier()

        TileContext._drain_and_barrier = _split_drain_and_barrier
        TileContext._drain_patch_applied = True

    if not getattr(bu, "_ldw_opt_patch_applied", False):
        orig = bu.get_walrus_args

        def _walrus_args_ldw_opt(*a, **kw):
            return [
                arg.replace("--enable-ldw-opt=false", "--enable-ldw-opt=true")
                for arg in orig(*a, **kw)
            ]

        bu.get_walrus_args = _walrus_args_ldw_opt
        bu._ldw_opt_patch_applied = True


def _split_multi_waits(nc, max_waits=1):
    """Move surplus semaphore waits onto single-wait nops inserted just before
    the owning instruction on the same engine (the sequencer executes them in
    order, so the guarded instruction still issues only after all waits)."""
    import concourse.mybir as mybir

    k = 0
    for f in nc.m.functions:
        for b in f.blocks:
            il = list(b.instructions)
            new = []
            changed = False
            for inst in il:
                si = inst.sync_info
                waits = list(si.on_wait) if si is not None else []
                if len(waits) > max_waits:
                    changed = True
                    for w in waits[:-max_waits]:
                        nop = mybir.InstNoOp(name=f"Wsplit-{k}", ins=[], outs=[])
                        k += 1
                        nop.engine = inst.engine
                        nop.sync_info = mybir.SyncInfo(on_wait=[w], on_update=[])
                        new.append(nop)
                    inst.sync_info = mybir.SyncInfo(
                        on_wait=waits[-max_waits:], on_update=list(si.on_update)
                    )
                new.append(inst)
            if changed:
                b.instructions = new


def _window_pieces(w0):
    """Split the hw window [w0, w0+128) into pieces that neither wrap 3136 nor
    cross a 512-wide PSUM bank boundary. Returns (dst_hw, src_col, width)."""
    if w0 + 128 <= HW:
        segs = [(w0, 0, 128)]
    else:
        r = HW - w0
        segs = [(w0, 0, r), (0, r, 128 - r)]
    out = []
    for d, s, n in segs:
        while n > 0:
            m = min(n, 512 - (d % 512))
            out.append((d, s, m))
            d += m
            s += m
            n -= m
    return out


def _build_nc():
    import concourse.bass as bass
    import concourse.mybir as mybir
    from concourse.tile import TileContext

    _apply_env_patches()

    f32, f16 = mybir.dt.float32, mybir.dt.float16
    nc = bass.Bass()

    xt = nc.dram_tensor("xt", [128, T * CW], f16, kind="ExternalInput")
    ic_d = nc.dram_tensor("ic16", [128, 128], f16, kind="ExternalInput")
    pk_d = nc.dram_tensor("pack", [128, _PACKW], f16, kind="ExternalInput")
    out_d = nc.dram_tensor("out", [128, HW], f32, kind="ExternalOutput")

    writes_per_bank = [0] * 7
    for t in range(T):
        for d, s, n in _window_pieces((128 * t) % HW):
            writes_per_bank[d // 512] += 1

    with TileContext(nc) as tc:
        with (
            tc.tile_pool(name="consts", bufs=1) as consts,
            tc.tile_pool(name="xtiles", bufs=3) as xtiles,
            tc.tile_pool(name="sbres", bufs=1) as sbres,
        ):
            # first x tile + the two const loads go out immediately; x tiles
            # alternate between the two HWDGE queues (sync / scalar).
            TW = TPJ * CW
            xt_sb0 = xtiles.tile([128, TW], f16, name="xt_sb0", tag="xt")
            nc.sync.dma_start(out=xt_sb0[:], in_=xt[:, 0:TW])
            ic_sb = consts.tile([128, 128], f16)
            nc.scalar.dma_start(out=ic_sb[:], in_=ic_d[:])
            pk_sb = consts.tile([128, _PACKW], f16)
            nc.scalar.dma_start(out=pk_sb[:], in_=pk_d[:])

            wqT_sb = pk_sb[:, _WQ : _WQ + 128]
            wkT_sb = pk_sb[:, _WK : _WK + 128]
            wv_sb = pk_sb[:, _WV : _WV + 128]
            woT_sb = pk_sb[:, _WO : _WO + 128]
            id_sb = pk_sb[:, _ID : _ID + 128]
            bv_sb = pk_sb[:, _BV : _BV + 1]
            bo_sb = pk_sb[:, _BO : _BO + 1]
            bq_sb = pk_sb[0:1, _BQ : _BQ + 128]
            bk_sb = pk_sb[0:1, _BK : _BK + 128]

            # ---- pass 1: [G|s] and x_mean, one sweep over x^T chunks ----
            with tc.tile_pool(name="ps1", bufs=1, space="PSUM") as ps1:
                g_ps = ps1.tile([128, CW], f32)
                xm_ps = [
                    ps1.tile([128, 512], f32, name=f"xm{k}", tag=f"xm{k}")
                    for k in range(6)
                ]
                xm_ps.append(ps1.tile([128, 64], f32, name="xm6", tag="xm6"))

                seen_per_bank = [0] * 7
                for j in range(J):
                    if j == 0:
                        xt_sb = xt_sb0
                    else:
                        xt_sb = xtiles.tile(
                            [128, TW], f16, name=f"xt_sb{j}", tag="xt"
                        )
                        eng = nc.sync if j % 2 == 0 else nc.scalar
                        eng.dma_start(
                            out=xt_sb[:], in_=xt[:, TW * j : TW * (j + 1)]
                        )
                    for i in range(TPJ):
                        t = TPJ * j + i
                        ch = xt_sb[:, CW * i : CW * i + 128]
                        nc.tensor.matmul(
                            g_ps[:],
                            lhsT=ch,
                            rhs=xt_sb[:, CW * i : CW * i + CW],
                            start=(t == 0),
                            stop=(t == T - 1),
                        )
                        for d, s, n in _window_pieces((128 * t) % HW):
                            bk_i = d // 512
                            seen_per_bank[bk_i] += 1
                            nc.tensor.matmul(
                                xm_ps[bk_i][:, d % 512 : d % 512 + n],
                                lhsT=ch,
                                rhs=ic_sb[:, s : s + n],
                                start=(seen_per_bank[bk_i] == 1),
                                stop=(seen_per_bank[bk_i] == writes_per_bank[bk_i]),
                            )

                # drain [G|s] then x_mean to SBUF (vector engine)
                gs_sb = sbres.tile([128, CW], f16)
                nc.vector.tensor_copy(out=gs_sb[:], in_=g_ps[:])
                xm_sb = sbres.tile([128, HW], f16)
                for k in range(7):
                    wdt = 64 if k == 6 else 512
                    nc.vector.tensor_copy(
                        out=xm_sb[:, 512 * k : 512 * k + wdt], in_=xm_ps[k][:]
                    )

            # ---- pass 2: serial tail (reuses the pass-1 PSUM banks) ----
            with tc.tile_pool(name="ps2", bufs=1, space="PSUM") as ps2:
                if True:
                    g_sb = gs_sb[:, 0:128]
                    s_col = gs_sb[:, 128:129]

                    # s^T and (Wk s)^T as 1-partition rows
                    rows_ps = ps2.tile([1, 512], f32, tag="sm")
                    nc.tensor.matmul(
                        rows_ps[:, 0:128], lhsT=s_col, rhs=id_sb,
                        start=True, stop=True,
                    )
                    nc.tensor.matmul(
                        rows_ps[:, 128:256], lhsT=s_col, rhs=wkT_sb,
                        start=True, stop=True, skip_group_check=True,
                    )
                    srow_sb = sbres.tile([1, 128], f16)
                    nc.vector.tensor_copy(out=srow_sb[:], in_=rows_ps[:, 0:128])
                    kkrow_sb = sbres.tile([1, 128], f16)
                    nc.vector.tensor_copy(out=kkrow_sb[:], in_=rows_ps[:, 128:256])

                    # r2 = L*bk + (Wk s)^T   [1,128]
                    r2_sb = sbres.tile([1, 128], f16)
                    nc.vector.scalar_tensor_tensor(
                        out=r2_sb[:],
                        in0=bk_sb,
                        scalar=float(L),
                        in1=kkrow_sb[:],
                        op0=mybir.AluOpType.mult,
                        op1=mybir.AluOpType.add,
                    )

                    # V1 = G Wk^T + s bk^T
                    v1_ps = ps2.tile([128, 128], f32, tag="mm")
                    nc.tensor.matmul(
                        v1_ps[:], lhsT=g_sb, rhs=wkT_sb, start=True, stop=False
                    )
                    nc.tensor.matmul(
                        v1_ps[:], lhsT=srow_sb[:], rhs=bk_sb, start=False, stop=True
                    )
                    v1_sb = sbres.tile([128, 128], f16)
                    nc.vector.tensor_copy(out=v1_sb[:], in_=v1_ps[:])

                    # logits = Wq V1 + bq (outer) r2
                    lg_ps = ps2.tile([128, 128], f32, tag="mm2")
                    nc.tensor.matmul(
                        lg_ps[:], lhsT=wqT_sb, rhs=v1_sb[:], start=True, stop=False
                    )
                    nc.tensor.matmul(
                        lg_ps[:], lhsT=bq_sb, rhs=r2_sb[:], start=False, stop=True
                    )

                    # softmax over the free axis (ACT only does the exp)
                    negmax = sbres.tile([128, 1], f32)
                    nc.vector.tensor_reduce(
                        out=negmax[:], in_=lg_ps[:], axis=mybir.AxisListType.X,
                        op=mybir.AluOpType.max, negate=True,
                    )
                    a_sb = sbres.tile([128, 128], f16)
                    sumexp = sbres.tile([128, 1], f32)
                    nc.scalar.activation(
                        out=a_sb[:], in_=lg_ps[:],
                        func=mybir.ActivationFunctionType.Exp,
                        bias=negmax[:], scale=1.0, accum_out=sumexp[:],
                    )
                    rec = sbres.tile([128, 1], f32)
                    nc.vector.reciprocal(out=rec[:], in_=sumexp[:])
                    nc.vector.tensor_scalar_mul(a_sb[:], a_sb[:], rec[:])

                    # U = A^T Wo^T  [b, o]
                    u_ps = ps2.tile([128, 128], f32, tag="mm")
                    nc.tensor.matmul(
                        u_ps[:], lhsT=a_sb[:], rhs=woT_sb, start=True, stop=True
                    )
                    u_sb = sbres.tile([128, 128], f16)
                    nc.vector.tensor_copy(out=u_sb[:], in_=u_ps[:])

                    # M^T = Wv^T A^T Wo^T  (the +I rides the out matmuls)
                    mt_ps = ps2.tile([128, 128], f32, tag="mm2")
                    nc.tensor.matmul(
                        mt_ps[:], lhsT=wv_sb, rhs=u_sb[:], start=True, stop=True
                    )
                    mt_sb = sbres.tile([128, 128], f16)
                    nc.vector.tensor_copy(out=mt_sb[:], in_=mt_ps[:])

                    # cvec = U^T bv + bo  [o,1]
                    cv_ps = ps2.tile([128, 64], f32, tag="sm")
                    nc.tensor.matmul(
                        cv_ps[:, 0:1], lhsT=u_sb[:], rhs=bv_sb,
                        start=True, stop=True, skip_group_check=True,
                    )
                    cvec_sb = sbres.tile([128, 1], f32)
                    nc.vector.scalar_tensor_tensor(
                        out=cvec_sb[:],
                        in0=cv_ps[:, 0:1],
                        scalar=1.0,
                        in1=bo_sb,
                        op0=mybir.AluOpType.mult,
                        op1=mybir.AluOpType.add,
                    )

                    # out = (M + I) x_mean + cvec, 7 chunks of 448 columns;
                    # bias-adds alternate DVE/ACT, DMA per chunk on both queues
                    out_sb = sbres.tile([128, HW], f32)
                    for k in range(7):
                        oc_ps = ps2.tile([128, 448], f32, name=f"oc{k}", tag="oc", bufs=2)
                        nc.tensor.matmul(
                            oc_ps[:],
                            lhsT=mt_sb[:],
                            rhs=xm_sb[:, 448 * k : 448 * (k + 1)],
                            start=True, stop=False,
                        )
                        nc.tensor.matmul(
                            oc_ps[:],
                            lhsT=id_sb,
                            rhs=xm_sb[:, 448 * k : 448 * (k + 1)],
                            start=False, stop=True,
                        )
                        ob = out_sb[:, 448 * k : 448 * (k + 1)]
                        if k % 2 == 0:
                            nc.vector.tensor_scalar_add(ob, oc_ps[:], cvec_sb[:])
                        else:
                            nc.scalar.activation(
                                out=ob, in_=oc_ps[:],
                                func=mybir.ActivationFunctionType.Identity,
                                bias=cvec_sb[:], scale=1.0,
                            )
                        eng = nc.sync if k % 2 == 0 else nc.scalar
                        eng.dma_start(
                            out=out_d[:, 448 * k : 448 * (k + 1)], in_=ob
                        )

    _split_multi_waits(nc)
    return nc


_cached_nc = None


def kernel(x, w_q, b_q, w_k, b_k, w_v, b_v, w_o, b_o):
    global _cached_nc, _last_results
    from concourse.bass_utils import run_bass_kernel_spmd

    if _cached_nc is None:
        _cached_nc = _build_nc()
    nc = _cached_nc

    x = np.asarray(x, np.float32)
    pack = np.zeros((128, _PACKW), np.float16)
    pack[:, _WQ : _WQ + 128] = np.asarray(w_q, np.float32).T.astype(np.float16)
    pack[:, _WK : _WK + 128] = np.asarray(w_k, np.float32).T.astype(np.float16)
    pack[:, _WV : _WV + 128] = np.asarray(w_v, np.float32).astype(np.float16)
    pack[:, _WO : _WO + 128] = np.asarray(w_o, np.float32).T.astype(np.float16)
    pack[:, _ID : _ID + 128] = np.eye(128, dtype=np.float16)
    pack[:, _BV] = np.asarray(b_v, np.float16)
    pack[:, _BO] = np.asarray(b_o, np.float16)
    pack[0, _BQ : _BQ + 128] = np.asarray(b_q, np.float16)
    pack[0, _BK : _BK + 128] = np.asarray(b_k, np.float16)
    ic16 = np.ascontiguousarray((np.eye(128) / 16.0).astype(np.float16))

    in_maps = []
    for b in range(B):
        # xt[p, CW*t + c] = x[b, c, 128*t + p] for c < 128; ones at c == 128
        xb = x[b].reshape(C, T, 128)
        xt_b = np.empty((128, T, CW), np.float16)
        xt_b[:, :, :128] = xb.transpose(2, 1, 0).astype(np.float16)
        xt_b[:, :, 128] = np.float16(1.0)
        in_maps.append(
            {"xt": xt_b.reshape(128, T * CW), "ic16": ic16, "pack": pack}
        )

    res = run_bass_kernel_spmd(nc, in_maps, list(range(N_CORES)))
    _last_results = res

    out = np.empty((B, C, H, W), np.float32)
    for b in range(B):
        out[b] = res.results[b]["out"].reshape(C, H, W)
    return out


# revision 8
# speedup vs baseline: 1.3926x; 1.0236x over previous
"""Channel self-attention module (CSMA) on 8 Trainium2 NeuronCores.

Math: with x [B,C,N,H,W], C==HID==OUT==128, L=N*H*W, the module is
    q = Wq x + bq ; k = Wk x + bk ; v = Wv x + bv          (per-batch [C,L])
    A = softmax(q k^T)                                     ([C,C], rows)
    out = Wo (A v) + bo + x ; result = mean_N(out)         ([C,H*W])

Everything except the softmax is linear in x, so per batch only two small
sufficient statistics of x are needed:
    G = x x^T  [C,C]   and   s = x 1_L  [C]
    logits = Wq G Wk^T + (Wq s) bk^T + bq (Wk s)^T + L bq bk^T
    A = softmax(logits)
    result = (Wo A Wv + I) x_mean + (Wo A bv + bo)
where x_mean = mean over N of x (shape [C, H*W]).

Device pass 1 computes G, s and x_mean in one sweep: x is pre-transposed on
the host to l-major fp16 chunks [128l, 128c] with a ones column appended per
chunk; each chunk is the stationary operand for (a) a [chunk|ones] stream
accumulating [G|s] and (b) a (1/16)-identity stream accumulating the chunk
into the x_mean PSUM window for its hw range. Pass 2 is a short serial tail
of [128,128]-scale fp16 matmuls + softmax, with PSUM drains on the otherwise
idle vector engine.

Sharding: data-parallel over batch — core b handles batch element b.
"""

import numpy as np

B, C, N, H, W = 8, 128, 16, 56, 56
HW = H * W            # 3136
L = N * HW            # 50176
T = L // 128          # 392 chunks of 128 l-values
TPJ = 28              # chunks per DMA tile
J = T // TPJ          # 14 DMA tiles
CW = 129              # chunk width in the xt layout (128 cols + ones column)
N_CORES = 8

# fp16 const-pack column layout
_WQ, _WK, _WV, _WO, _ID = 0, 128, 256, 384, 512
_BV, _BO, _BQ, _BK = 640, 641, 642, 770
_PACKW = 898

_last_results = None  # BassKernelResults of the most recent run (for profiling)


def _apply_env_patches():
    """Workarounds for this container's walrus build.

    1. Tile's end-of-kernel Drain aggregates every outstanding sem wait onto
       one CTRL instruction, but this walrus rejects >1 wait per instruction
       ("Too many sync wait commands"): re-emit surplus waits as single-wait
       nops (see _split_multi_waits, applied post-build).
    2. --enable-ldw-opt=true lets codegen skip redundant LDWEIGHTS reloads
       for consecutive matmuls sharing a stationary operand.
    """
    import concourse.mybir as mybir
    import concourse.bass_utils as bu
    from concourse.tile import TileContext
    from concourse.vector_clock import ScopedClock

    if not getattr(TileContext, "_drain_patch_applied", False):

        def _split_drain_and_barrier(self, tick_clock, wait_clock):
            # All end-of-kernel waits go on GpSimd — the engine that then
            # clears the semaphores — so the clear cannot pass an in-flight
            # producer. The two all-engine barriers are dropped: every
            # engine's stream simply ends, and the runtime's completion
            # signal requires all engines (including GpSimd) to halt.
            probe = self.nc.gpsimd.nop(nofuse=True)
            wait_clock.add_sem_waits(
                probe.ins, ScopedClock({None: tick_clock.global_clock})
            )
            si = probe.ins.sync_info
            waits = list(si.on_wait) if si is not None else []
            if len(waits) > 1:
                probe.ins.sync_info = mybir.SyncInfo(
                    on_wait=waits[:1], on_update=list(si.on_update)
                )
                for w in waits[1:]:
                    n = self.nc.gpsimd.nop(nofuse=True)
                    n.ins.sync_info = mybir.SyncInfo(on_wait=[w], on_update=[])
            assert self.sems is not None
            popped = self.nc._tile_sem_poison_stack.pop()
            assert popped is self._sem_poison
            self.nc.clear_and_free_semaphores(list(self.sems.allocated().values()))

        TileContext._drain_and_barrier = _split_drain_and_barrier
        TileContext._drain_patch_applied = True

    if not getattr(bu, "_ldw_opt_patch_applied", False):
        orig = bu.get_walrus_args

        def _walrus_args_ldw_opt(*a, **kw):
            return [
                arg.replace("--enable-ldw-opt=false", "--enable-ldw-opt=true")
                for arg in orig(*a, **kw)
            ]

        bu.get_walrus_args = _walrus_args_ldw_opt
        bu._ldw_opt_patch_applied = True


def _split_multi_waits(nc, max_waits=1):
    """Move surplus semaphore waits onto single-wait nops inserted just before
    the owning instruction on the same engine (the sequencer executes them in
    order, so the guarded instruction still issues only after all waits)."""
    import concourse.mybir as mybir

    k = 0
    for f in nc.m.functions:
        for b in f.blocks:
            il = list(b.instructions)
            new = []
            changed = False
            for inst in il:
                si = inst.sync_info
                waits = list(si.on_wait) if si is not None else []
                if len(waits) > max_waits:
                    changed = True
                    for w in waits[:-max_waits]:
                        nop = mybir.InstNoOp(name=f"Wsplit-{k}", ins=[], outs=[])
                        k += 1
                        nop.engine = inst.engine
                        nop.sync_info = mybir.SyncInfo(on_wait=[w], on_update=[])
                        new.append(nop)
                    inst.sync_info = mybir.SyncInfo(
                        on_wait=waits[-max_waits:], on_update=list(si.on_update)
                    )
                new.append(inst)
            if changed:
                b.instructions = new



def _hoist_first_dmas(nc, n=3):
    """Move the first wait-free DMA loads (first x tile + the const packs)
    ahead of the framework's entry barriers so the HBM transfers overlap the
    ~7 us engine-init prologue."""
    for f in nc.m.functions:
        for b in f.blocks:
            il = list(b.instructions)
            dmas = []
            for i in il:
                if i.opcode == "DMACopy":
                    si = i.sync_info
                    if si is None or not si.on_wait:
                        dmas.append(i)
                    if len(dmas) >= n:
                        break
            if not dmas:
                continue
            picked = set(id(x) for x in dmas)
            rest = [i for i in il if id(i) not in picked]
            b.instructions = rest[:1] + dmas + rest[1:]


def _window_pieces(w0):
    """Split the hw window [w0, w0+128) into pieces that neither wrap 3136 nor
    cross a 512-wide PSUM bank boundary. Returns (dst_hw, src_col, width)."""
    if w0 + 128 <= HW:
        segs = [(w0, 0, 128)]
    else:
        r = HW - w0
        segs = [(w0, 0, r), (0, r, 128 - r)]
    out = []
    for d, s, n in segs:
        while n > 0:
            m = min(n, 512 - (d % 512))
            out.append((d, s, m))
            d += m
            s += m
            n -= m
    return out


def _build_nc():
    import concourse.bass as bass
    import concourse.mybir as mybir
    from concourse.tile import TileContext

    _apply_env_patches()

    f32, f16 = mybir.dt.float32, mybir.dt.float16
    nc = bass.Bass()

    xt = nc.dram_tensor("xt", [128, T * CW], f16, kind="ExternalInput")
    ic_d = nc.dram_tensor("ic16", [128, 128], f16, kind="ExternalInput")
    pk_d = nc.dram_tensor("pack", [128, _PACKW], f16, kind="ExternalInput")
    out_d = nc.dram_tensor("out", [128, HW], f32, kind="ExternalOutput")

    writes_per_bank = [0] * 7
    for t in range(T):
        for d, s, n in _window_pieces((128 * t) % HW):
            writes_per_bank[d // 512] += 1

    with TileContext(nc) as tc:
        with (
            tc.tile_pool(name="consts", bufs=1) as consts,
            tc.tile_pool(name="xtiles", bufs=5) as xtiles,
            tc.tile_pool(name="sbres", bufs=1) as sbres,
        ):
            # first x tile + the two const loads go out immediately; x tiles
            # alternate between the two HWDGE queues (sync / scalar).
            TW = TPJ * CW
            xt_sb0 = xtiles.tile([128, TW], f16, name="xt_sb0", tag="xt")
            nc.sync.dma_start(out=xt_sb0[:], in_=xt[:, 0:TW])
            ic_sb = consts.tile([128, 128], f16)
            nc.scalar.dma_start(out=ic_sb[:], in_=ic_d[:])
            pk_sb = consts.tile([128, _PACKW], f16)
            nc.scalar.dma_start(out=pk_sb[:], in_=pk_d[:])

            wqT_sb = pk_sb[:, _WQ : _WQ + 128]
            wkT_sb = pk_sb[:, _WK : _WK + 128]
            wv_sb = pk_sb[:, _WV : _WV + 128]
            woT_sb = pk_sb[:, _WO : _WO + 128]
            id_sb = pk_sb[:, _ID : _ID + 128]
            bv_sb = pk_sb[:, _BV : _BV + 1]
            bo_sb = pk_sb[:, _BO : _BO + 1]
            bq_sb = pk_sb[0:1, _BQ : _BQ + 128]
            bk_sb = pk_sb[0:1, _BK : _BK + 128]

            # ---- pass 1: [G|s] and x_mean, one sweep over x^T chunks ----
            with tc.tile_pool(name="ps1", bufs=1, space="PSUM") as ps1:
                g_ps = ps1.tile([128, CW], f32)
                xm_ps = [
                    ps1.tile([128, 512], f32, name=f"xm{k}", tag=f"xm{k}")
                    for k in range(6)
                ]
                xm_ps.append(ps1.tile([128, 64], f32, name="xm6", tag="xm6"))

                seen_per_bank = [0] * 7
                for j in range(J):
                    if j == 0:
                        xt_sb = xt_sb0
                    else:
                        xt_sb = xtiles.tile(
                            [128, TW], f16, name=f"xt_sb{j}", tag="xt"
                        )
                        eng = nc.sync if j % 2 == 0 else nc.scalar
                        eng.dma_start(
                            out=xt_sb[:], in_=xt[:, TW * j : TW * (j + 1)]
                        )
                    for i in range(TPJ):
                        t = TPJ * j + i
                        ch = xt_sb[:, CW * i : CW * i + 128]
                        nc.tensor.matmul(
                            g_ps[:],
                            lhsT=ch,
                            rhs=xt_sb[:, CW * i : CW * i + CW],
                            start=(t == 0),
                            stop=(t == T - 1),
                        )
                        for d, s, n in _window_pieces((128 * t) % HW):
                            bk_i = d // 512
                            seen_per_bank[bk_i] += 1
                            nc.tensor.matmul(
                                xm_ps[bk_i][:, d % 512 : d % 512 + n],
                                lhsT=ch,
                                rhs=ic_sb[:, s : s + n],
                                start=(seen_per_bank[bk_i] == 1),
                                stop=(seen_per_bank[bk_i] == writes_per_bank[bk_i]),
                            )

                # drain [G|s] then x_mean to SBUF (vector engine)
                gs_sb = sbres.tile([128, CW], f16)
                nc.vector.tensor_copy(out=gs_sb[:], in_=g_ps[:])
                xm_sb = sbres.tile([128, HW], f16)
                for k in range(7):
                    wdt = 64 if k == 6 else 512
                    nc.vector.tensor_copy(
                        out=xm_sb[:, 512 * k : 512 * k + wdt], in_=xm_ps[k][:]
                    )

            # ---- pass 2: serial tail (reuses the pass-1 PSUM banks) ----
            with tc.tile_pool(name="ps2", bufs=1, space="PSUM") as ps2:
                if True:
                    g_sb = gs_sb[:, 0:128]
                    s_col = gs_sb[:, 128:129]

                    # s^T and (Wk s)^T as 1-partition rows
                    rows_ps = ps2.tile([1, 512], f32, tag="sm")
                    nc.tensor.matmul(
                        rows_ps[:, 0:128], lhsT=s_col, rhs=id_sb,
                        start=True, stop=True,
                    )
                    nc.tensor.matmul(
                        rows_ps[:, 128:256], lhsT=s_col, rhs=wkT_sb,
                        start=True, stop=True, skip_group_check=True,
                    )
                    srow_sb = sbres.tile([1, 128], f16)
                    nc.vector.tensor_copy(out=srow_sb[:], in_=rows_ps[:, 0:128])
                    kkrow_sb = sbres.tile([1, 128], f16)
                    nc.vector.tensor_copy(out=kkrow_sb[:], in_=rows_ps[:, 128:256])

                    # r2 = L*bk + (Wk s)^T   [1,128]
                    r2_sb = sbres.tile([1, 128], f16)
                    nc.vector.scalar_tensor_tensor(
                        out=r2_sb[:],
                        in0=bk_sb,
                        scalar=float(L),
                        in1=kkrow_sb[:],
                        op0=mybir.AluOpType.mult,
                        op1=mybir.AluOpType.add,
                    )

                    # V1 = G Wk^T + s bk^T
                    v1_ps = ps2.tile([128, 128], f32, tag="mm")
                    nc.tensor.matmul(
                        v1_ps[:], lhsT=g_sb, rhs=wkT_sb, start=True, stop=False
                    )
                    nc.tensor.matmul(
                        v1_ps[:], lhsT=srow_sb[:], rhs=bk_sb, start=False, stop=True
                    )
                    v1_sb = sbres.tile([128, 128], f16)
                    nc.vector.tensor_copy(out=v1_sb[:], in_=v1_ps[:])

                    # logits = Wq V1 + bq (outer) r2
                    lg_ps = ps2.tile([128, 128], f32, tag="mm2")
                    nc.tensor.matmul(
                        lg_ps[:], lhsT=wqT_sb, rhs=v1_sb[:], start=True, stop=False
                    )
                    nc.tensor.matmul(
                        lg_ps[:], lhsT=bq_sb, rhs=r2_sb[:], start=False, stop=True
                    )

                    # softmax over the free axis (ACT only does the exp)
                    negmax = sbres.tile([128, 1], f32)
                    nc.vector.tensor_reduce(
                        out=negmax[:], in_=lg_ps[:], axis=mybir.AxisListType.X,
                        op=mybir.AluOpType.max, negate=True,
                    )
                    a_sb = sbres.tile([128, 128], f16)
                    sumexp = sbres.tile([128, 1], f32)
                    nc.scalar.activation(
                        out=a_sb[:], in_=lg_ps[:],
                        func=mybir.ActivationFunctionType.Exp,
                        bias=negmax[:], scale=1.0, accum_out=sumexp[:],
                    )
                    rec = sbres.tile([128, 1], f32)
                    nc.vector.reciprocal(out=rec[:], in_=sumexp[:])
                    nc.vector.tensor_scalar_mul(a_sb[:], a_sb[:], rec[:])

                    # U = A^T Wo^T  [b, o]
                    u_ps = ps2.tile([128, 128], f32, tag="mm")
                    nc.tensor.matmul(
                        u_ps[:], lhsT=a_sb[:], rhs=woT_sb, start=True, stop=True
                    )
                    u_sb = sbres.tile([128, 128], f16)
                    nc.vector.tensor_copy(out=u_sb[:], in_=u_ps[:])

                    # M^T = Wv^T A^T Wo^T  (the +I rides the out matmuls)
                    mt_ps = ps2.tile([128, 128], f32, tag="mm2")
                    nc.tensor.matmul(
                        mt_ps[:], lhsT=wv_sb, rhs=u_sb[:], start=True, stop=True
                    )
                    mt_sb = sbres.tile([128, 128], f16)
                    nc.vector.tensor_copy(out=mt_sb[:], in_=mt_ps[:])

                    # cvec = U^T bv + bo  [o,1]
                    cv_ps = ps2.tile([128, 64], f32, tag="sm")
                    nc.tensor.matmul(
                        cv_ps[:, 0:1], lhsT=u_sb[:], rhs=bv_sb,
                        start=True, stop=True, skip_group_check=True,
                    )
                    cvec_sb = sbres.tile([128, 1], f32)
                    nc.vector.scalar_tensor_tensor(
                        out=cvec_sb[:],
                        in0=cv_ps[:, 0:1],
                        scalar=1.0,
                        in1=bo_sb,
                        op0=mybir.AluOpType.mult,
                        op1=mybir.AluOpType.add,
                    )

                    # out = (M + I) x_mean + cvec, 7 chunks of 448 columns;
                    # bias-adds alternate DVE/ACT, DMA per chunk on both queues
                    out_sb = sbres.tile([128, HW], f32)
                    for k in range(7):
                        oc_ps = ps2.tile([128, 448], f32, name=f"oc{k}", tag="oc", bufs=2)
                        nc.tensor.matmul(
                            oc_ps[:],
                            lhsT=mt_sb[:],
                            rhs=xm_sb[:, 448 * k : 448 * (k + 1)],
                            start=True, stop=False,
                        )
                        nc.tensor.matmul(
                            oc_ps[:],
                            lhsT=id_sb,
                            rhs=xm_sb[:, 448 * k : 448 * (k + 1)],
                            start=False, stop=True,
                        )
                        ob = out_sb[:, 448 * k : 448 * (k + 1)]
                        if k % 2 == 0:
                            nc.vector.tensor_scalar_add(ob, oc_ps[:], cvec_sb[:])
                        else:
                            nc.scalar.activation(
                                out=ob, in_=oc_ps[:],
                                func=mybir.ActivationFunctionType.Identity,
                                bias=cvec_sb[:], scale=1.0,
                            )
                        eng = nc.sync if k % 2 == 0 else nc.scalar
                        eng.dma_start(
                            out=out_d[:, 448 * k : 448 * (k + 1)], in_=ob
                        )

    _split_multi_waits(nc)
    _hoist_first_dmas(nc)
    return nc


_cached_nc = None


def kernel(x, w_q, b_q, w_k, b_k, w_v, b_v, w_o, b_o):
    global _cached_nc, _last_results
    from concourse.bass_utils import run_bass_kernel_spmd

    if _cached_nc is None:
        _cached_nc = _build_nc()
    nc = _cached_nc

    x = np.asarray(x, np.float32)
    pack = np.zeros((128, _PACKW), np.float16)
    pack[:, _WQ : _WQ + 128] = np.asarray(w_q, np.float32).T.astype(np.float16)
    pack[:, _WK : _WK + 128] = np.asarray(w_k, np.float32).T.astype(np.float16)
    pack[:, _WV : _WV + 128] = np.asarray(w_v, np.float32).astype(np.float16)
    pack[:, _WO : _WO + 128] = np.asarray(w_o, np.float32).T.astype(np.float16)
    pack[:, _ID : _ID + 128] = np.eye(128, dtype=np.float16)
    pack[:, _BV] = np.asarray(b_v, np.float16)
    pack[:, _BO] = np.asarray(b_o, np.float16)
    pack[0, _BQ : _BQ + 128] = np.asarray(b_q, np.float16)
    pack[0, _BK : _BK + 128] = np.asarray(b_k, np.float16)
    ic16 = np.ascontiguousarray((np.eye(128) / 16.0).astype(np.float16))

    in_maps = []
    for b in range(B):
        # xt[p, CW*t + c] = x[b, c, 128*t + p] for c < 128; ones at c == 128
        xb = x[b].reshape(C, T, 128)
        xt_b = np.empty((128, T, CW), np.float16)
        xt_b[:, :, :128] = xb.transpose(2, 1, 0).astype(np.float16)
        xt_b[:, :, 128] = np.float16(1.0)
        in_maps.append(
            {"xt": xt_b.reshape(128, T * CW), "ic16": ic16, "pack": pack}
        )

    res = run_bass_kernel_spmd(nc, in_maps, list(range(N_CORES)))
    _last_results = res

    out = np.empty((B, C, H, W), np.float32)
    for b in range(B):
        out[b] = res.results[b]["out"].reshape(C, H, W)
    return out


# revision 10
# speedup vs baseline: 1.3967x; 1.0029x over previous
"""Channel self-attention module (CSMA) on 8 Trainium2 NeuronCores.

Math: with x [B,C,N,H,W], C==HID==OUT==128, L=N*H*W, the module is
    q = Wq x + bq ; k = Wk x + bk ; v = Wv x + bv          (per-batch [C,L])
    A = softmax(q k^T)                                     ([C,C], rows)
    out = Wo (A v) + bo + x ; result = mean_N(out)         ([C,H*W])

Everything except the softmax is linear in x, so per batch only two small
sufficient statistics of x are needed:
    G = x x^T  [C,C]   and   s = x 1_L  [C]
    logits = Wq G Wk^T + (Wq s) bk^T + bq (Wk s)^T + L bq bk^T
    A = softmax(logits)
    result = (Wo A Wv + I) x_mean + (Wo A bv + bo)
where x_mean = mean over N of x (shape [C, H*W]).

Device pass 1 computes G, s and x_mean in one sweep: x is pre-transposed on
the host to l-major fp16 chunks [128l, 128c] with a ones column appended per
chunk; each chunk is the stationary operand for (a) a [chunk|ones] stream
accumulating [G|s] and (b) a (1/16)-identity stream accumulating the chunk
into the x_mean PSUM window for its hw range. Pass 2 is a short serial tail
of [128,128]-scale fp16 matmuls + softmax, with PSUM drains on the otherwise
idle vector engine.

Sharding: data-parallel over batch — core b handles batch element b.
"""

import numpy as np

B, C, N, H, W = 8, 128, 16, 56, 56
HW = H * W            # 3136
L = N * HW            # 50176
T = L // 128          # 392 chunks of 128 l-values
TPJ = 28              # chunks per DMA tile
J = T // TPJ          # 14 DMA tiles
CW = 129              # chunk width in the xt layout (128 cols + ones column)
N_CORES = 8

# fp16 const-pack column layout
_WQ, _WK, _WV, _WO, _ID = 0, 128, 256, 384, 512
_BV, _BO, _BQ, _BK = 640, 641, 642, 770
_PACKW = 898

_last_results = None  # BassKernelResults of the most recent run (for profiling)


def _apply_env_patches():
    """Workarounds for this container's walrus build.

    1. Tile's end-of-kernel Drain aggregates every outstanding sem wait onto
       one CTRL instruction, but this walrus rejects >1 wait per instruction
       ("Too many sync wait commands"): re-emit surplus waits as single-wait
       nops (see _split_multi_waits, applied post-build).
    2. --enable-ldw-opt=true lets codegen skip redundant LDWEIGHTS reloads
       for consecutive matmuls sharing a stationary operand.
    """
    import concourse.mybir as mybir
    import concourse.bass_utils as bu
    from concourse.tile import TileContext
    from concourse.vector_clock import ScopedClock

    if not getattr(TileContext, "_drain_patch_applied", False):

        def _split_drain_and_barrier(self, tick_clock, wait_clock):
            # All end-of-kernel waits go on GpSimd — the engine that then
            # clears the semaphores — so the clear cannot pass an in-flight
            # producer. The two all-engine barriers are dropped: every
            # engine's stream simply ends, and the runtime's completion
            # signal requires all engines (including GpSimd) to halt.
            probe = self.nc.gpsimd.nop(nofuse=True)
            wait_clock.add_sem_waits(
                probe.ins, ScopedClock({None: tick_clock.global_clock})
            )
            si = probe.ins.sync_info
            waits = list(si.on_wait) if si is not None else []
            if len(waits) > 1:
                probe.ins.sync_info = mybir.SyncInfo(
                    on_wait=waits[:1], on_update=list(si.on_update)
                )
                for w in waits[1:]:
                    n = self.nc.gpsimd.nop(nofuse=True)
                    n.ins.sync_info = mybir.SyncInfo(on_wait=[w], on_update=[])
            assert self.sems is not None
            popped = self.nc._tile_sem_poison_stack.pop()
            assert popped is self._sem_poison
            self.nc.clear_and_free_semaphores(list(self.sems.allocated().values()))

        TileContext._drain_and_barrier = _split_drain_and_barrier
        TileContext._drain_patch_applied = True

    if not getattr(bu, "_ldw_opt_patch_applied", False):
        orig = bu.get_walrus_args

        def _walrus_args_ldw_opt(*a, **kw):
            return [
                arg.replace("--enable-ldw-opt=false", "--enable-ldw-opt=true")
                for arg in orig(*a, **kw)
            ]

        bu.get_walrus_args = _walrus_args_ldw_opt
        bu._ldw_opt_patch_applied = True


def _split_multi_waits(nc, max_waits=1):
    """Move surplus semaphore waits onto single-wait nops inserted just before
    the owning instruction on the same engine (the sequencer executes them in
    order, so the guarded instruction still issues only after all waits)."""
    import concourse.mybir as mybir

    k = 0
    for f in nc.m.functions:
        for b in f.blocks:
            il = list(b.instructions)
            new = []
            changed = False
            for inst in il:
                si = inst.sync_info
                waits = list(si.on_wait) if si is not None else []
                if len(waits) > max_waits:
                    changed = True
                    for w in waits[:-max_waits]:
                        nop = mybir.InstNoOp(name=f"Wsplit-{k}", ins=[], outs=[])
                        k += 1
                        nop.engine = inst.engine
                        nop.sync_info = mybir.SyncInfo(on_wait=[w], on_update=[])
                        new.append(nop)
                    inst.sync_info = mybir.SyncInfo(
                        on_wait=waits[-max_waits:], on_update=list(si.on_update)
                    )
                new.append(inst)
            if changed:
                b.instructions = new



def _hoist_first_dmas(nc, n=3):
    """Move the first wait-free DMA loads (first x tile + the const packs)
    from the tile-context block into the entry block, ahead of the framework's
    engine-init barriers, so the HBM transfers overlap the ~7 us prologue."""
    for f in nc.m.functions:
        blocks = list(f.blocks)
        if len(blocks) < 2:
            continue
        entry, body = blocks[0], blocks[1]
        bil = list(body.instructions)
        dmas = []
        for i in bil:
            if i.opcode == "DMACopy":
                si = i.sync_info
                if si is None or not si.on_wait:
                    dmas.append(i)
                if len(dmas) >= n:
                    break
        if not dmas:
            continue
        picked = set(id(x) for x in dmas)
        body.instructions = [i for i in bil if id(i) not in picked]
        eil = list(entry.instructions)
        entry.instructions = eil[:1] + dmas + eil[1:]


def _window_pieces(w0):
    """Split the hw window [w0, w0+128) into pieces that neither wrap 3136 nor
    cross a 512-wide PSUM bank boundary. Returns (dst_hw, src_col, width)."""
    if w0 + 128 <= HW:
        segs = [(w0, 0, 128)]
    else:
        r = HW - w0
        segs = [(w0, 0, r), (0, r, 128 - r)]
    out = []
    for d, s, n in segs:
        while n > 0:
            m = min(n, 512 - (d % 512))
            out.append((d, s, m))
            d += m
            s += m
            n -= m
    return out


def _build_nc():
    import concourse.bass as bass
    import concourse.mybir as mybir
    from concourse.tile import TileContext

    _apply_env_patches()

    f32, f16 = mybir.dt.float32, mybir.dt.float16
    nc = bass.Bass()

    xt = nc.dram_tensor("xt", [128, T * CW], f16, kind="ExternalInput")
    ic_d = nc.dram_tensor("ic16", [128, 128], f16, kind="ExternalInput")
    pk_d = nc.dram_tensor("pack", [128, _PACKW], f16, kind="ExternalInput")
    out_d = nc.dram_tensor("out", [128, HW], f32, kind="ExternalOutput")

    writes_per_bank = [0] * 7
    for t in range(T):
        for d, s, n in _window_pieces((128 * t) % HW):
            writes_per_bank[d // 512] += 1

    with TileContext(nc) as tc:
        with (
            tc.tile_pool(name="consts", bufs=1) as consts,
            tc.tile_pool(name="xtiles", bufs=5) as xtiles,
            tc.tile_pool(name="sbres", bufs=1) as sbres,
        ):
            # first x tiles are small so PE starts early; x tiles alternate
            # between the two HWDGE queues (sync / scalar).
            tile_chunks = [14, 14] + [TPJ] * ((T - 28) // TPJ)
            assert sum(tile_chunks) == T
            xt_sb0 = xtiles.tile([128, 14 * CW], f16, name="xt_sb0", tag="xt0")
            nc.sync.dma_start(out=xt_sb0[:], in_=xt[:, 0 : 14 * CW])
            ic_sb = consts.tile([128, 128], f16)
            nc.scalar.dma_start(out=ic_sb[:], in_=ic_d[:])
            warm = sbres.tile([1, 1], f32)
            nc.vector.memset(warm[:], 0.0)
            nc.scalar.activation(
                out=warm[:], in_=warm[:],
                func=mybir.ActivationFunctionType.Exp, bias=0.0, scale=1.0,
            )
            pk_sb = consts.tile([128, _PACKW], f16)
            nc.scalar.dma_start(out=pk_sb[:], in_=pk_d[:])

            wqT_sb = pk_sb[:, _WQ : _WQ + 128]
            wkT_sb = pk_sb[:, _WK : _WK + 128]
            wv_sb = pk_sb[:, _WV : _WV + 128]
            woT_sb = pk_sb[:, _WO : _WO + 128]
            id_sb = pk_sb[:, _ID : _ID + 128]
            bv_sb = pk_sb[:, _BV : _BV + 1]
            bo_sb = pk_sb[:, _BO : _BO + 1]
            bq_sb = pk_sb[0:1, _BQ : _BQ + 128]
            bk_sb = pk_sb[0:1, _BK : _BK + 128]

            # ---- pass 1: [G|s] and x_mean, one sweep over x^T chunks ----
            with tc.tile_pool(name="ps1", bufs=1, space="PSUM") as ps1:
                g_ps = ps1.tile([128, CW], f32)
                xm_ps = [
                    ps1.tile([128, 512], f32, name=f"xm{k}", tag=f"xm{k}")
                    for k in range(6)
                ]
                xm_ps.append(ps1.tile([128, 64], f32, name="xm6", tag="xm6"))

                seen_per_bank = [0] * 7
                t0 = 0
                for j, ntc in enumerate(tile_chunks):
                    if j == 0:
                        xt_sb = xt_sb0
                    else:
                        xt_sb = xtiles.tile(
                            [128, ntc * CW], f16, name=f"xt_sb{j}",
                            tag="xt0" if j == 1 else "xt",
                        )
                        eng = nc.sync if j % 2 == 0 else nc.scalar
                        eng.dma_start(
                            out=xt_sb[:],
                            in_=xt[:, CW * t0 : CW * (t0 + ntc)],
                        )
                    for i in range(ntc):
                        t = t0 + i
                        ch = xt_sb[:, CW * i : CW * i + 128]
                        nc.tensor.matmul(
                            g_ps[:],
                            lhsT=ch,
                            rhs=xt_sb[:, CW * i : CW * i + CW],
                            start=(t == 0),
                            stop=(t == T - 1),
                        )
                        for d, s, n in _window_pieces((128 * t) % HW):
                            bk_i = d // 512
                            seen_per_bank[bk_i] += 1
                            nc.tensor.matmul(
                                xm_ps[bk_i][:, d % 512 : d % 512 + n],
                                lhsT=ch,
                                rhs=ic_sb[:, s : s + n],
                                start=(seen_per_bank[bk_i] == 1),
                                stop=(seen_per_bank[bk_i] == writes_per_bank[bk_i]),
                            )
                    t0 += ntc

                # drain [G|s] then x_mean to SBUF (vector engine; narrow
                # copies so the G drain is never stuck behind a long one)
                gs_sb = sbres.tile([128, CW], f16)
                nc.vector.tensor_copy(out=gs_sb[:], in_=g_ps[:])
                xm_sb = sbres.tile([128, HW], f16)
                for k in range(7):
                    wdt = 64 if k == 6 else 512
                    for o in range(0, wdt, 256):
                        w = min(256, wdt - o)
                        nc.vector.tensor_copy(
                            out=xm_sb[:, 512 * k + o : 512 * k + o + w],
                            in_=xm_ps[k][:, o : o + w],
                        )

            # ---- pass 2: serial tail (reuses the pass-1 PSUM banks) ----
            with tc.tile_pool(name="ps2", bufs=1, space="PSUM") as ps2:
                if True:
                    g_sb = gs_sb[:, 0:128]
                    s_col = gs_sb[:, 128:129]

                    # s^T and (Wk s)^T as 1-partition rows
                    rows_ps = ps2.tile([1, 512], f32, tag="sm")
                    nc.tensor.matmul(
                        rows_ps[:, 0:128], lhsT=s_col, rhs=id_sb,
                        start=True, stop=True,
                    )
                    nc.tensor.matmul(
                        rows_ps[:, 128:256], lhsT=s_col, rhs=wkT_sb,
                        start=True, stop=True, skip_group_check=True,
                    )
                    rows_sb = sbres.tile([1, 256], f16)
                    nc.vector.tensor_copy(out=rows_sb[:], in_=rows_ps[:, 0:256])
                    srow_sb = rows_sb[:, 0:128]
                    kkrow_sb = rows_sb[:, 128:256]

                    # r2 = L*bk + (Wk s)^T   [1,128]
                    r2_sb = sbres.tile([1, 128], f16)
                    nc.vector.scalar_tensor_tensor(
                        out=r2_sb[:],
                        in0=bk_sb,
                        scalar=float(L),
                        in1=kkrow_sb,
                        op0=mybir.AluOpType.mult,
                        op1=mybir.AluOpType.add,
                    )

                    # V1 = G Wk^T + s bk^T
                    v1_ps = ps2.tile([128, 128], f32, tag="mm")
                    nc.tensor.matmul(
                        v1_ps[:], lhsT=g_sb, rhs=wkT_sb, start=True, stop=False
                    )
                    nc.tensor.matmul(
                        v1_ps[:], lhsT=srow_sb, rhs=bk_sb, start=False, stop=True
                    )
                    v1_sb = sbres.tile([128, 128], f16)
                    nc.vector.tensor_copy(out=v1_sb[:], in_=v1_ps[:])

                    # logits = Wq V1 + bq (outer) r2
                    lg_ps = ps2.tile([128, 128], f32, tag="mm2")
                    nc.tensor.matmul(
                        lg_ps[:], lhsT=wqT_sb, rhs=v1_sb[:], start=True, stop=False
                    )
                    nc.tensor.matmul(
                        lg_ps[:], lhsT=bq_sb, rhs=r2_sb[:], start=False, stop=True
                    )

                    # softmax over the free axis (ACT only does the exp)
                    negmax = sbres.tile([128, 1], f32)
                    nc.vector.tensor_reduce(
                        out=negmax[:], in_=lg_ps[:], axis=mybir.AxisListType.X,
                        op=mybir.AluOpType.max, negate=True,
                    )
                    a_sb = sbres.tile([128, 128], f16)
                    sumexp = sbres.tile([128, 1], f32)
                    nc.scalar.activation(
                        out=a_sb[:], in_=lg_ps[:],
                        func=mybir.ActivationFunctionType.Exp,
                        bias=negmax[:], scale=1.0, accum_out=sumexp[:],
                    )
                    rec = sbres.tile([128, 1], f32)
                    nc.vector.reciprocal(out=rec[:], in_=sumexp[:])
                    nc.vector.tensor_scalar_mul(a_sb[:], a_sb[:], rec[:])

                    # U = A^T Wo^T  [b, o]
                    u_ps = ps2.tile([128, 128], f32, tag="mm")
                    nc.tensor.matmul(
                        u_ps[:], lhsT=a_sb[:], rhs=woT_sb, start=True, stop=True
                    )
                    u_sb = sbres.tile([128, 128], f16)
                    nc.vector.tensor_copy(out=u_sb[:], in_=u_ps[:])

                    # M^T = Wv^T A^T Wo^T ; P^T = M^T + I
                    mt_ps = ps2.tile([128, 128], f32, tag="mm2")
                    nc.tensor.matmul(
                        mt_ps[:], lhsT=wv_sb, rhs=u_sb[:], start=True, stop=True
                    )
                    pt_sb = sbres.tile([128, 128], f16)
                    nc.vector.tensor_add(out=pt_sb[:], in0=mt_ps[:], in1=id_sb)

                    # cvec = U^T bv + bo  [o,1]
                    cv_ps = ps2.tile([128, 64], f32, tag="sm")
                    nc.tensor.matmul(
                        cv_ps[:, 0:1], lhsT=u_sb[:], rhs=bv_sb,
                        start=True, stop=True, skip_group_check=True,
                    )
                    cvec_sb = sbres.tile([128, 1], f32)
                    nc.vector.scalar_tensor_tensor(
                        out=cvec_sb[:],
                        in0=cv_ps[:, 0:1],
                        scalar=1.0,
                        in1=bo_sb,
                        op0=mybir.AluOpType.mult,
                        op1=mybir.AluOpType.add,
                    )

                    # out = (M + I) x_mean + cvec, 7 chunks of 448 columns;
                    # bias-adds alternate DVE/ACT, DMA per chunk on both queues
                    out_sb = sbres.tile([128, HW], f32)
                    for k in range(7):
                        oc_ps = ps2.tile([128, 448], f32, name=f"oc{k}", tag="oc", bufs=3)
                        nc.tensor.matmul(
                            oc_ps[:],
                            lhsT=pt_sb[:],
                            rhs=xm_sb[:, 448 * k : 448 * (k + 1)],
                            start=True, stop=True,
                        )
                        ob = out_sb[:, 448 * k : 448 * (k + 1)]
                        if k % 2 == 0:
                            nc.vector.tensor_scalar_add(ob, oc_ps[:], cvec_sb[:])
                        else:
                            nc.scalar.activation(
                                out=ob, in_=oc_ps[:],
                                func=mybir.ActivationFunctionType.Identity,
                                bias=cvec_sb[:], scale=1.0,
                            )
                        eng = nc.sync if k % 2 == 0 else nc.scalar
                        eng.dma_start(
                            out=out_d[:, 448 * k : 448 * (k + 1)], in_=ob
                        )

    _split_multi_waits(nc)
    _hoist_first_dmas(nc)
    return nc


_cached_nc = None


def kernel(x, w_q, b_q, w_k, b_k, w_v, b_v, w_o, b_o):
    global _cached_nc, _last_results
    from concourse.bass_utils import run_bass_kernel_spmd

    if _cached_nc is None:
        _cached_nc = _build_nc()
    nc = _cached_nc

    x = np.asarray(x, np.float32)
    pack = np.zeros((128, _PACKW), np.float16)
    pack[:, _WQ : _WQ + 128] = np.asarray(w_q, np.float32).T.astype(np.float16)
    pack[:, _WK : _WK + 128] = np.asarray(w_k, np.float32).T.astype(np.float16)
    pack[:, _WV : _WV + 128] = np.asarray(w_v, np.float32).astype(np.float16)
    pack[:, _WO : _WO + 128] = np.asarray(w_o, np.float32).T.astype(np.float16)
    pack[:, _ID : _ID + 128] = np.eye(128, dtype=np.float16)
    pack[:, _BV] = np.asarray(b_v, np.float16)
    pack[:, _BO] = np.asarray(b_o, np.float16)
    pack[0, _BQ : _BQ + 128] = np.asarray(b_q, np.float16)
    pack[0, _BK : _BK + 128] = np.asarray(b_k, np.float16)
    ic16 = np.ascontiguousarray((np.eye(128) / 16.0).astype(np.float16))

    in_maps = []
    for b in range(B):
        # xt[p, CW*t + c] = x[b, c, 128*t + p] for c < 128; ones at c == 128
        xb = x[b].reshape(C, T, 128)
        xt_b = np.empty((128, T, CW), np.float16)
        xt_b[:, :, :128] = xb.transpose(2, 1, 0).astype(np.float16)
        xt_b[:, :, 128] = np.float16(1.0)
        in_maps.append(
            {"xt": xt_b.reshape(128, T * CW), "ic16": ic16, "pack": pack}
        )

    res = run_bass_kernel_spmd(nc, in_maps, list(range(N_CORES)))
    _last_results = res

    out = np.empty((B, C, H, W), np.float32)
    for b in range(B):
        out[b] = res.results[b]["out"].reshape(C, H, W)
    return out


# revision 11
# speedup vs baseline: 1.4250x; 1.0203x over previous
"""Channel self-attention module (CSMA) on 8 Trainium2 NeuronCores.

Math: with x [B,C,N,H,W], C==HID==OUT==128, L=N*H*W, the module is
    q = Wq x + bq ; k = Wk x + bk ; v = Wv x + bv          (per-batch [C,L])
    A = softmax(q k^T)                                     ([C,C], rows)
    out = Wo (A v) + bo + x ; result = mean_N(out)         ([C,H*W])

Everything except the softmax is linear in x, so per batch only two small
sufficient statistics of x are needed:
    G = x x^T  [C,C]   and   s = x 1_L  [C]
    logits = Wq G Wk^T + (Wq s) bk^T + bq (Wk s)^T + L bq bk^T
    A = softmax(logits)
    result = (Wo A Wv + I) x_mean + (Wo A bv + bo)
where x_mean = mean over N of x (shape [C, H*W]).

Device pass 1 computes G, s and x_mean in one sweep: x is pre-transposed on
the host to l-major fp16 chunks [128l, 128c] with a ones column appended per
chunk; each chunk is the stationary operand for (a) a [chunk|ones] stream
accumulating [G|s] and (b) a (1/16)-identity stream accumulating the chunk
into the x_mean PSUM window for its hw range. Pass 2 is a short serial tail
of [128,128]-scale fp16 matmuls + softmax, with PSUM drains on the otherwise
idle vector engine.

Sharding: data-parallel over batch — core b handles batch element b.
"""

import numpy as np

B, C, N, H, W = 8, 128, 16, 56, 56
HW = H * W            # 3136
L = N * HW            # 50176
T = L // 128          # 392 chunks of 128 l-values
TPJ = 28              # chunks per DMA tile
J = T // TPJ          # 14 DMA tiles
CW = 129              # chunk width in the xt layout (128 cols + ones column)
N_CORES = 8

# fp16 const-pack column layout
_WQ, _WK, _WV, _WO, _ID = 0, 128, 256, 384, 512
_BV, _BO, _BQ, _BK = 640, 641, 642, 770
_PACKW = 898

_last_results = None  # BassKernelResults of the most recent run (for profiling)


def _apply_env_patches():
    """Workarounds for this container's walrus build.

    1. Tile's end-of-kernel Drain aggregates every outstanding sem wait onto
       one CTRL instruction, but this walrus rejects >1 wait per instruction
       ("Too many sync wait commands"): re-emit surplus waits as single-wait
       nops (see _split_multi_waits, applied post-build).
    2. --enable-ldw-opt=true lets codegen skip redundant LDWEIGHTS reloads
       for consecutive matmuls sharing a stationary operand.
    """
    import concourse.mybir as mybir
    import concourse.bass_utils as bu
    from concourse.tile import TileContext
    from concourse.vector_clock import ScopedClock

    if not getattr(TileContext, "_drain_patch_applied", False):

        def _split_drain_and_barrier(self, tick_clock, wait_clock):
            # All end-of-kernel waits go on GpSimd — the engine that then
            # clears the semaphores — so the clear cannot pass an in-flight
            # producer. The two all-engine barriers are dropped: every
            # engine's stream simply ends, and the runtime's completion
            # signal requires all engines (including GpSimd) to halt.
            probe = self.nc.gpsimd.nop(nofuse=True)
            wait_clock.add_sem_waits(
                probe.ins, ScopedClock({None: tick_clock.global_clock})
            )
            si = probe.ins.sync_info
            waits = list(si.on_wait) if si is not None else []
            if len(waits) > 1:
                probe.ins.sync_info = mybir.SyncInfo(
                    on_wait=waits[:1], on_update=list(si.on_update)
                )
                for w in waits[1:]:
                    n = self.nc.gpsimd.nop(nofuse=True)
                    n.ins.sync_info = mybir.SyncInfo(on_wait=[w], on_update=[])
            assert self.sems is not None
            popped = self.nc._tile_sem_poison_stack.pop()
            assert popped is self._sem_poison
            self.nc.clear_and_free_semaphores(list(self.sems.allocated().values()))

        TileContext._drain_and_barrier = _split_drain_and_barrier
        TileContext._drain_patch_applied = True

    if not getattr(bu, "_ldw_opt_patch_applied", False):
        orig = bu.get_walrus_args

        def _walrus_args_ldw_opt(*a, **kw):
            return [
                arg.replace("--enable-ldw-opt=false", "--enable-ldw-opt=true")
                for arg in orig(*a, **kw)
            ]

        bu.get_walrus_args = _walrus_args_ldw_opt
        bu._ldw_opt_patch_applied = True


def _split_multi_waits(nc, max_waits=1):
    """Move surplus semaphore waits onto single-wait nops inserted just before
    the owning instruction on the same engine (the sequencer executes them in
    order, so the guarded instruction still issues only after all waits)."""
    import concourse.mybir as mybir

    k = 0
    for f in nc.m.functions:
        for b in f.blocks:
            il = list(b.instructions)
            new = []
            changed = False
            for inst in il:
                si = inst.sync_info
                waits = list(si.on_wait) if si is not None else []
                if len(waits) > max_waits:
                    changed = True
                    for w in waits[:-max_waits]:
                        nop = mybir.InstNoOp(name=f"Wsplit-{k}", ins=[], outs=[])
                        k += 1
                        nop.engine = inst.engine
                        nop.sync_info = mybir.SyncInfo(on_wait=[w], on_update=[])
                        new.append(nop)
                    inst.sync_info = mybir.SyncInfo(
                        on_wait=waits[-max_waits:], on_update=list(si.on_update)
                    )
                new.append(inst)
            if changed:
                b.instructions = new



def _hoist_first_dmas(nc, n=3):
    """Move the first wait-free DMA loads (first x tile + the const packs)
    from the tile-context block into the entry block, ahead of the framework's
    engine-init barriers, so the HBM transfers overlap the ~7 us prologue."""
    for f in nc.m.functions:
        blocks = list(f.blocks)
        if len(blocks) < 2:
            continue
        entry, body = blocks[0], blocks[1]
        bil = list(body.instructions)
        dmas = []
        for i in bil:
            if i.opcode == "DMACopy":
                si = i.sync_info
                if si is None or not si.on_wait:
                    dmas.append(i)
                if len(dmas) >= n:
                    break
        if not dmas:
            continue
        picked = set(id(x) for x in dmas)
        body.instructions = [i for i in bil if id(i) not in picked]
        eil = list(entry.instructions)
        entry.instructions = eil[:1] + dmas + eil[1:]


def _window_pieces(w0):
    """Split the hw window [w0, w0+128) into pieces that neither wrap 3136 nor
    cross a 512-wide PSUM bank boundary. Returns (dst_hw, src_col, width)."""
    if w0 + 128 <= HW:
        segs = [(w0, 0, 128)]
    else:
        r = HW - w0
        segs = [(w0, 0, r), (0, r, 128 - r)]
    out = []
    for d, s, n in segs:
        while n > 0:
            m = min(n, 512 - (d % 512))
            out.append((d, s, m))
            d += m
            s += m
            n -= m
    return out


def _build_nc():
    import concourse.bass as bass
    import concourse.mybir as mybir
    from concourse.tile import TileContext

    _apply_env_patches()

    f32, f16 = mybir.dt.float32, mybir.dt.float16
    nc = bass.Bass()

    xt = nc.dram_tensor("xt", [128, T * CW], f16, kind="ExternalInput")
    ic_d = nc.dram_tensor("ic16", [128, 128], f16, kind="ExternalInput")
    pk_d = nc.dram_tensor("pack", [128, _PACKW], f16, kind="ExternalInput")
    out_d = nc.dram_tensor("out", [128, HW], f32, kind="ExternalOutput")

    writes_per_bank = [0] * 7
    for t in range(T):
        for d, s, n in _window_pieces((128 * t) % HW):
            writes_per_bank[d // 512] += 1

    with TileContext(nc) as tc:
        with (
            tc.tile_pool(name="consts", bufs=1) as consts,
            tc.tile_pool(name="xtiles", bufs=5) as xtiles,
            tc.tile_pool(name="sbres", bufs=1) as sbres,
        ):
            # x tiles alternate between the two HWDGE queues (sync / scalar)
            tile_chunks = [TPJ] * J
            xt_sb0 = xtiles.tile([128, TPJ * CW], f16, name="xt_sb0", tag="xt")
            nc.sync.dma_start(out=xt_sb0[:], in_=xt[:, 0 : TPJ * CW])
            ic_sb = consts.tile([128, 128], f16)
            nc.scalar.dma_start(out=ic_sb[:], in_=ic_d[:])
            warm = sbres.tile([1, 1], f32)
            nc.vector.memset(warm[:], 0.0)
            nc.scalar.activation(
                out=warm[:], in_=warm[:],
                func=mybir.ActivationFunctionType.Exp, bias=0.0, scale=1.0,
            )
            pk_sb = consts.tile([128, _PACKW], f16)
            nc.scalar.dma_start(out=pk_sb[:], in_=pk_d[:])

            wqT_sb = pk_sb[:, _WQ : _WQ + 128]
            wkT_sb = pk_sb[:, _WK : _WK + 128]
            wv_sb = pk_sb[:, _WV : _WV + 128]
            woT_sb = pk_sb[:, _WO : _WO + 128]
            id_sb = pk_sb[:, _ID : _ID + 128]
            bv_sb = pk_sb[:, _BV : _BV + 1]
            bo_sb = pk_sb[:, _BO : _BO + 1]
            bq_sb = pk_sb[0:1, _BQ : _BQ + 128]
            bk_sb = pk_sb[0:1, _BK : _BK + 128]

            # ---- pass 1: [G|s] and x_mean, one sweep over x^T chunks ----
            with tc.tile_pool(name="ps1", bufs=1, space="PSUM") as ps1:
                g_ps = ps1.tile([128, CW], f32)
                xm_ps = [
                    ps1.tile([128, 512], f32, name=f"xm{k}", tag=f"xm{k}")
                    for k in range(6)
                ]
                xm_ps.append(ps1.tile([128, 64], f32, name="xm6", tag="xm6"))

                seen_per_bank = [0] * 7
                t0 = 0
                for j, ntc in enumerate(tile_chunks):
                    if j == 0:
                        xt_sb = xt_sb0
                    else:
                        xt_sb = xtiles.tile(
                            [128, ntc * CW], f16, name=f"xt_sb{j}", tag="xt"
                        )
                        eng = nc.sync if j % 2 == 0 else nc.scalar
                        eng.dma_start(
                            out=xt_sb[:],
                            in_=xt[:, CW * t0 : CW * (t0 + ntc)],
                        )
                    for i in range(ntc):
                        t = t0 + i
                        ch = xt_sb[:, CW * i : CW * i + 128]
                        nc.tensor.matmul(
                            g_ps[:],
                            lhsT=ch,
                            rhs=xt_sb[:, CW * i : CW * i + CW],
                            start=(t == 0),
                            stop=(t == T - 1),
                        )
                        for d, s, n in _window_pieces((128 * t) % HW):
                            bk_i = d // 512
                            seen_per_bank[bk_i] += 1
                            nc.tensor.matmul(
                                xm_ps[bk_i][:, d % 512 : d % 512 + n],
                                lhsT=ch,
                                rhs=ic_sb[:, s : s + n],
                                start=(seen_per_bank[bk_i] == 1),
                                stop=(seen_per_bank[bk_i] == writes_per_bank[bk_i]),
                            )
                    t0 += ntc

                # drain [G|s] then x_mean to SBUF (vector engine; narrow
                # copies so the G drain is never stuck behind a long one)
                gs_sb = sbres.tile([128, CW], f16)
                nc.vector.tensor_copy(out=gs_sb[:], in_=g_ps[:])
                xm_sb = sbres.tile([128, HW], f16)
                for k in range(7):
                    wdt = 64 if k == 6 else 512
                    for o in range(0, wdt, 256):
                        w = min(256, wdt - o)
                        nc.vector.tensor_copy(
                            out=xm_sb[:, 512 * k + o : 512 * k + o + w],
                            in_=xm_ps[k][:, o : o + w],
                        )

            # ---- pass 2: serial tail (reuses the pass-1 PSUM banks) ----
            with tc.tile_pool(name="ps2", bufs=1, space="PSUM") as ps2:
                if True:
                    g_sb = gs_sb[:, 0:128]
                    s_col = gs_sb[:, 128:129]

                    # s^T and (Wk s)^T as 1-partition rows
                    rows_ps = ps2.tile([1, 512], f32, tag="sm")
                    nc.tensor.matmul(
                        rows_ps[:, 0:128], lhsT=s_col, rhs=id_sb,
                        start=True, stop=True,
                    )
                    nc.tensor.matmul(
                        rows_ps[:, 128:256], lhsT=s_col, rhs=wkT_sb,
                        start=True, stop=True, skip_group_check=True,
                    )
                    rows_sb = sbres.tile([1, 256], f16)
                    nc.vector.tensor_copy(out=rows_sb[:], in_=rows_ps[:, 0:256])
                    srow_sb = rows_sb[:, 0:128]
                    kkrow_sb = rows_sb[:, 128:256]

                    # r2 = L*bk + (Wk s)^T   [1,128]
                    r2_sb = sbres.tile([1, 128], f16)
                    nc.vector.scalar_tensor_tensor(
                        out=r2_sb[:],
                        in0=bk_sb,
                        scalar=float(L),
                        in1=kkrow_sb,
                        op0=mybir.AluOpType.mult,
                        op1=mybir.AluOpType.add,
                    )

                    # V1 = G Wk^T + s bk^T
                    v1_ps = ps2.tile([128, 128], f32, tag="mm")
                    nc.tensor.matmul(
                        v1_ps[:], lhsT=g_sb, rhs=wkT_sb, start=True, stop=False
                    )
                    nc.tensor.matmul(
                        v1_ps[:], lhsT=srow_sb, rhs=bk_sb, start=False, stop=True
                    )
                    v1_sb = sbres.tile([128, 128], f16)
                    nc.vector.tensor_copy(out=v1_sb[:], in_=v1_ps[:])

                    # logits = Wq V1 + bq (outer) r2
                    lg_ps = ps2.tile([128, 128], f32, tag="mm2")
                    nc.tensor.matmul(
                        lg_ps[:], lhsT=wqT_sb, rhs=v1_sb[:], start=True, stop=False
                    )
                    nc.tensor.matmul(
                        lg_ps[:], lhsT=bq_sb, rhs=r2_sb[:], start=False, stop=True
                    )

                    # softmax over the free axis (ACT only does the exp)
                    negmax = sbres.tile([128, 1], f32)
                    nc.vector.tensor_reduce(
                        out=negmax[:], in_=lg_ps[:], axis=mybir.AxisListType.X,
                        op=mybir.AluOpType.max, negate=True,
                    )
                    a_sb = sbres.tile([128, 128], f16)
                    sumexp = sbres.tile([128, 1], f32)
                    nc.scalar.activation(
                        out=a_sb[:], in_=lg_ps[:],
                        func=mybir.ActivationFunctionType.Exp,
                        bias=negmax[:], scale=1.0, accum_out=sumexp[:],
                    )
                    rec = sbres.tile([128, 1], f32)
                    nc.vector.reciprocal(out=rec[:], in_=sumexp[:])
                    nc.vector.tensor_scalar_mul(a_sb[:], a_sb[:], rec[:])

                    # U = A^T Wo^T  [b, o]
                    u_ps = ps2.tile([128, 128], f32, tag="mm")
                    nc.tensor.matmul(
                        u_ps[:], lhsT=a_sb[:], rhs=woT_sb, start=True, stop=True
                    )
                    u_sb = sbres.tile([128, 128], f16)
                    nc.vector.tensor_copy(out=u_sb[:], in_=u_ps[:])

                    # M^T = Wv^T A^T Wo^T ; P^T = M^T + I
                    mt_ps = ps2.tile([128, 128], f32, tag="mm2")
                    nc.tensor.matmul(
                        mt_ps[:], lhsT=wv_sb, rhs=u_sb[:], start=True, stop=True
                    )
                    pt_sb = sbres.tile([128, 128], f16)
                    nc.vector.tensor_add(out=pt_sb[:], in0=mt_ps[:], in1=id_sb)

                    # cvec = U^T bv + bo  [o,1]
                    cv_ps = ps2.tile([128, 64], f32, tag="sm")
                    nc.tensor.matmul(
                        cv_ps[:, 0:1], lhsT=u_sb[:], rhs=bv_sb,
                        start=True, stop=True, skip_group_check=True,
                    )
                    cvec_sb = sbres.tile([128, 1], f32)
                    nc.vector.scalar_tensor_tensor(
                        out=cvec_sb[:],
                        in0=cv_ps[:, 0:1],
                        scalar=1.0,
                        in1=bo_sb,
                        op0=mybir.AluOpType.mult,
                        op1=mybir.AluOpType.add,
                    )

                    # out = (M + I) x_mean + cvec, 7 chunks of 448 columns;
                    # bias-adds alternate DVE/ACT, DMA per chunk on both queues
                    out_sb = sbres.tile([128, HW], f32)
                    for k in range(7):
                        oc_ps = ps2.tile([128, 448], f32, name=f"oc{k}", tag="oc", bufs=3)
                        nc.tensor.matmul(
                            oc_ps[:],
                            lhsT=pt_sb[:],
                            rhs=xm_sb[:, 448 * k : 448 * (k + 1)],
                            start=True, stop=True,
                        )
                        ob = out_sb[:, 448 * k : 448 * (k + 1)]
                        if k % 2 == 0:
                            nc.vector.tensor_scalar_add(ob, oc_ps[:], cvec_sb[:])
                        else:
                            nc.scalar.activation(
                                out=ob, in_=oc_ps[:],
                                func=mybir.ActivationFunctionType.Identity,
                                bias=cvec_sb[:], scale=1.0,
                            )
                        eng = nc.sync if k % 2 == 0 else nc.scalar
                        eng.dma_start(
                            out=out_d[:, 448 * k : 448 * (k + 1)], in_=ob
                        )

    _split_multi_waits(nc)
    _hoist_first_dmas(nc)
    return nc


_cached_nc = None


def kernel(x, w_q, b_q, w_k, b_k, w_v, b_v, w_o, b_o):
    global _cached_nc, _last_results
    from concourse.bass_utils import run_bass_kernel_spmd

    if _cached_nc is None:
        _cached_nc = _build_nc()
    nc = _cached_nc

    x = np.asarray(x, np.float32)
    pack = np.zeros((128, _PACKW), np.float16)
    pack[:, _WQ : _WQ + 128] = np.asarray(w_q, np.float32).T.astype(np.float16)
    pack[:, _WK : _WK + 128] = np.asarray(w_k, np.float32).T.astype(np.float16)
    pack[:, _WV : _WV + 128] = np.asarray(w_v, np.float32).astype(np.float16)
    pack[:, _WO : _WO + 128] = np.asarray(w_o, np.float32).T.astype(np.float16)
    pack[:, _ID : _ID + 128] = np.eye(128, dtype=np.float16)
    pack[:, _BV] = np.asarray(b_v, np.float16)
    pack[:, _BO] = np.asarray(b_o, np.float16)
    pack[0, _BQ : _BQ + 128] = np.asarray(b_q, np.float16)
    pack[0, _BK : _BK + 128] = np.asarray(b_k, np.float16)
    ic16 = np.ascontiguousarray((np.eye(128) / 16.0).astype(np.float16))

    in_maps = []
    for b in range(B):
        # xt[p, CW*t + c] = x[b, c, 128*t + p] for c < 128; ones at c == 128
        xb = x[b].reshape(C, T, 128)
        xt_b = np.empty((128, T, CW), np.float16)
        xt_b[:, :, :128] = xb.transpose(2, 1, 0).astype(np.float16)
        xt_b[:, :, 128] = np.float16(1.0)
        in_maps.append(
            {"xt": xt_b.reshape(128, T * CW), "ic16": ic16, "pack": pack}
        )

    res = run_bass_kernel_spmd(nc, in_maps, list(range(N_CORES)))
    _last_results = res

    out = np.empty((B, C, H, W), np.float32)
    for b in range(B):
        out[b] = res.results[b]["out"].reshape(C, H, W)
    return out


# revision 12
# speedup vs baseline: 1.6720x; 1.1733x over previous
"""Channel self-attention module (CSMA) on 8 Trainium2 NeuronCores.

Math: with x [B,C,N,H,W], C==HID==OUT==128, L=N*H*W, the module is
    q = Wq x + bq ; k = Wk x + bk ; v = Wv x + bv          (per-batch [C,L])
    A = softmax(q k^T)                                     ([C,C], rows)
    out = Wo (A v) + bo + x ; result = mean_N(out)         ([C,H*W])

Everything except the softmax is linear in x, so per batch only two small
sufficient statistics of x are needed:
    G = x x^T  [C,C]   and   s = x 1_L  [C]
    logits = Wq G Wk^T + (Wq s) bk^T + bq (Wk s)^T + L bq bk^T
    A = softmax(logits)
    result = (Wo A Wv + I) x_mean + (Wo A bv + bo)
where x_mean = mean over N of x (shape [C, H*W]).

Device pass 1 computes G, s and x_mean in one sweep: x is pre-transposed on
the host to l-major fp16 chunks [128l, 128c] with a ones column appended per
chunk; each chunk is the stationary operand for (a) a [chunk|ones] stream
accumulating [G|s] and (b) a (1/16)-identity stream accumulating the chunk
into the x_mean PSUM window for its hw range. Pass 2 is a short serial tail
of [128,128]-scale fp16 matmuls + softmax, with PSUM drains on the otherwise
idle vector engine.

Sharding: data-parallel over batch — core b handles batch element b.
"""

import numpy as np

B, C, N, H, W = 8, 128, 16, 56, 56
HW = H * W            # 3136
L = N * HW            # 50176
T = L // 128          # 392 chunks of 128 l-values
TPJ = 28              # chunks per DMA tile
J = T // TPJ          # 14 DMA tiles
CW = 130              # chunk width in xt layout (128 cols + ones + pad, 4B-aligned)
TF = T // 2           # 196 folded chunks for the x_mean stream
N_CORES = 8

# fp16 const-pack column layout
_WQ, _WK, _WV, _WO, _ID = 0, 128, 256, 384, 512
_BV, _BO, _BQ, _BK = 640, 641, 642, 770
_PACKW = 898

_last_results = None  # BassKernelResults of the most recent run (for profiling)


def _apply_env_patches():
    """Workarounds for this container's walrus build.

    1. Tile's end-of-kernel Drain aggregates every outstanding sem wait onto
       one CTRL instruction, but this walrus rejects >1 wait per instruction
       ("Too many sync wait commands"): re-emit surplus waits as single-wait
       nops (see _split_multi_waits, applied post-build).
    2. --enable-ldw-opt=true lets codegen skip redundant LDWEIGHTS reloads
       for consecutive matmuls sharing a stationary operand.
    """
    import concourse.mybir as mybir
    import concourse.bass_utils as bu
    from concourse.tile import TileContext
    from concourse.vector_clock import ScopedClock

    if not getattr(TileContext, "_drain_patch_applied", False):

        def _split_drain_and_barrier(self, tick_clock, wait_clock):
            # All end-of-kernel waits go on GpSimd — the engine that then
            # clears the semaphores — so the clear cannot pass an in-flight
            # producer. The two all-engine barriers are dropped: every
            # engine's stream simply ends, and the runtime's completion
            # signal requires all engines (including GpSimd) to halt.
            probe = self.nc.gpsimd.nop(nofuse=True)
            wait_clock.add_sem_waits(
                probe.ins, ScopedClock({None: tick_clock.global_clock})
            )
            si = probe.ins.sync_info
            waits = list(si.on_wait) if si is not None else []
            if len(waits) > 1:
                probe.ins.sync_info = mybir.SyncInfo(
                    on_wait=waits[:1], on_update=list(si.on_update)
                )
                for w in waits[1:]:
                    n = self.nc.gpsimd.nop(nofuse=True)
                    n.ins.sync_info = mybir.SyncInfo(on_wait=[w], on_update=[])
            assert self.sems is not None
            popped = self.nc._tile_sem_poison_stack.pop()
            assert popped is self._sem_poison
            self.nc.clear_and_free_semaphores(list(self.sems.allocated().values()))

        TileContext._drain_and_barrier = _split_drain_and_barrier
        TileContext._drain_patch_applied = True

    if not getattr(bu, "_ldw_opt_patch_applied", False):
        orig = bu.get_walrus_args

        def _walrus_args_ldw_opt(*a, **kw):
            return [
                arg.replace("--enable-ldw-opt=false", "--enable-ldw-opt=true")
                for arg in orig(*a, **kw)
            ]

        bu.get_walrus_args = _walrus_args_ldw_opt
        bu._ldw_opt_patch_applied = True


def _split_multi_waits(nc, max_waits=1):
    """Move surplus semaphore waits onto single-wait nops inserted just before
    the owning instruction on the same engine (the sequencer executes them in
    order, so the guarded instruction still issues only after all waits)."""
    import concourse.mybir as mybir

    k = 0
    for f in nc.m.functions:
        for b in f.blocks:
            il = list(b.instructions)
            new = []
            changed = False
            for inst in il:
                si = inst.sync_info
                waits = list(si.on_wait) if si is not None else []
                if len(waits) > max_waits:
                    changed = True
                    for w in waits[:-max_waits]:
                        nop = mybir.InstNoOp(name=f"Wsplit-{k}", ins=[], outs=[])
                        k += 1
                        nop.engine = inst.engine
                        nop.sync_info = mybir.SyncInfo(on_wait=[w], on_update=[])
                        new.append(nop)
                    inst.sync_info = mybir.SyncInfo(
                        on_wait=waits[-max_waits:], on_update=list(si.on_update)
                    )
                new.append(inst)
            if changed:
                b.instructions = new



def _hoist_first_dmas(nc, n=3):
    """Move the first wait-free DMA loads (first x tile + the const packs)
    from the tile-context block into the entry block, ahead of the framework's
    engine-init barriers, so the HBM transfers overlap the ~7 us prologue."""
    for f in nc.m.functions:
        blocks = list(f.blocks)
        if len(blocks) < 2:
            continue
        entry, body = blocks[0], blocks[1]
        bil = list(body.instructions)
        dmas = []
        for i in bil:
            if i.opcode == "DMACopy":
                si = i.sync_info
                if si is None or not si.on_wait:
                    dmas.append(i)
                if len(dmas) >= n:
                    break
        if not dmas:
            continue
        picked = set(id(x) for x in dmas)
        body.instructions = [i for i in bil if id(i) not in picked]
        eil = list(entry.instructions)
        entry.instructions = eil[:1] + dmas + eil[1:]


def _window_pieces(w0):
    """Split the hw window [w0, w0+128) into pieces that neither wrap 3136 nor
    cross a 512-wide PSUM bank boundary. Returns (dst_hw, src_col, width)."""
    if w0 + 128 <= HW:
        segs = [(w0, 0, 128)]
    else:
        r = HW - w0
        segs = [(w0, 0, r), (0, r, 128 - r)]
    out = []
    for d, s, n in segs:
        while n > 0:
            m = min(n, 512 - (d % 512))
            out.append((d, s, m))
            d += m
            s += m
            n -= m
    return out


def _build_nc():
    import concourse.bass as bass
    import concourse.mybir as mybir
    from concourse.tile import TileContext

    _apply_env_patches()

    f32, f16 = mybir.dt.float32, mybir.dt.float16
    nc = bass.Bass()

    xt = nc.dram_tensor("xt", [128, T * CW], f16, kind="ExternalInput")
    ic_d = nc.dram_tensor("ic16", [128, 128], f16, kind="ExternalInput")
    pk_d = nc.dram_tensor("pack", [128, _PACKW], f16, kind="ExternalInput")
    out_d = nc.dram_tensor("out", [128, HW], f32, kind="ExternalOutput")

    writes_per_bank = [0] * 7
    for k in range(T // 2):
        for d, s, n in _window_pieces((128 * k) % HW):
            writes_per_bank[d // 512] += 1

    with TileContext(nc) as tc:
        with (
            tc.tile_pool(name="consts", bufs=1) as consts,
            tc.tile_pool(name="xtiles", bufs=5) as xtiles,
            tc.tile_pool(name="sbres", bufs=1) as sbres,
        ):
            # x tiles alternate between the two HWDGE queues (sync / scalar)
            tile_chunks = [TPJ] * J
            xt_sb0 = xtiles.tile([128, TPJ * CW], f16, name="xt_sb0", tag="xt")
            nc.sync.dma_start(out=xt_sb0[:], in_=xt[:, 0 : TPJ * CW])
            ic_sb = consts.tile([128, 128], f16)
            nc.scalar.dma_start(out=ic_sb[:], in_=ic_d[:])
            warm = sbres.tile([1, 1], f32)
            nc.vector.memset(warm[:], 0.0)
            nc.scalar.activation(
                out=warm[:], in_=warm[:],
                func=mybir.ActivationFunctionType.Exp, bias=0.0, scale=1.0,
            )
            pk_sb = consts.tile([128, _PACKW], f16)
            nc.scalar.dma_start(out=pk_sb[:], in_=pk_d[:])

            wqT_sb = pk_sb[:, _WQ : _WQ + 128]
            wkT_sb = pk_sb[:, _WK : _WK + 128]
            wv_sb = pk_sb[:, _WV : _WV + 128]
            woT_sb = pk_sb[:, _WO : _WO + 128]
            id_sb = pk_sb[:, _ID : _ID + 128]
            bv_sb = pk_sb[:, _BV : _BV + 1]
            bo_sb = pk_sb[:, _BO : _BO + 1]
            bq_sb = pk_sb[0:1, _BQ : _BQ + 128]
            bk_sb = pk_sb[0:1, _BK : _BK + 128]

            # ---- pass 1: [G|s] over all chunks; x_mean over DVE-folded
            # chunk pairs (positions 2k / 2k+1 hold chunks k and k+196,
            # which share an x_mean window) ----
            with (
                tc.tile_pool(name="folds", bufs=3) as folds,
                tc.tile_pool(name="ps1", bufs=1, space="PSUM") as ps1,
            ):
                g_ps = ps1.tile([128, CW], f32)
                xm_ps = [
                    ps1.tile([128, 512], f32, name=f"xm{k}", tag=f"xm{k}")
                    for k in range(6)
                ]
                xm_ps.append(ps1.tile([128, 64], f32, name="xm6", tag="xm6"))

                seen_per_bank = [0] * 7
                t0 = 0
                for j, ntc in enumerate(tile_chunks):
                    if j == 0:
                        xt_sb = xt_sb0
                    else:
                        xt_sb = xtiles.tile(
                            [128, ntc * CW], f16, name=f"xt_sb{j}", tag="xt"
                        )
                        eng = nc.sync if j % 2 == 0 else nc.scalar
                        eng.dma_start(
                            out=xt_sb[:],
                            in_=xt[:, CW * t0 : CW * (t0 + ntc)],
                        )
                    for i in range(ntc):
                        p = t0 + i
                        ch = xt_sb[:, CW * i : CW * i + 128]
                        nc.tensor.matmul(
                            g_ps[:],
                            lhsT=ch,
                            rhs=xt_sb[:, CW * i : CW * i + CW],
                            start=(p == 0),
                            stop=(p == T - 1),
                        )
                    # fold the 2-chunk pairs: one strided fp16 add per tile
                    nf = ntc // 2
                    pairs = xt_sb[:].rearrange(
                        "q (k two c) -> q k two c", two=2, c=CW
                    )
                    fold_sb = folds.tile(
                        [128, nf, CW], f16, name=f"fold{j}", tag="fold"
                    )
                    nc.vector.tensor_add(
                        out=fold_sb[:],
                        in0=pairs[:, :, 0, :],
                        in1=pairs[:, :, 1, :],
                    )
                    for i in range(nf):
                        k = t0 // 2 + i
                        ch = fold_sb[:, i, 0:128]
                        for d, s, n in _window_pieces((128 * k) % HW):
                            bk_i = d // 512
                            seen_per_bank[bk_i] += 1
                            nc.tensor.matmul(
                                xm_ps[bk_i][:, d % 512 : d % 512 + n],
                                lhsT=ch,
                                rhs=ic_sb[:, s : s + n],
                                start=(seen_per_bank[bk_i] == 1),
                                stop=(seen_per_bank[bk_i] == writes_per_bank[bk_i]),
                            )
                    t0 += ntc

                # drain [G|s] then x_mean to SBUF (vector engine; narrow
                # copies so the G drain is never stuck behind a long one)
                gs_sb = sbres.tile([128, CW], f16)
                nc.vector.tensor_copy(out=gs_sb[:], in_=g_ps[:])
                xm_sb = sbres.tile([128, HW], f16)
                for k in range(7):
                    wdt = 64 if k == 6 else 512
                    for o in range(0, wdt, 256):
                        w = min(256, wdt - o)
                        nc.vector.tensor_copy(
                            out=xm_sb[:, 512 * k + o : 512 * k + o + w],
                            in_=xm_ps[k][:, o : o + w],
                        )

            # ---- pass 2: serial tail (reuses the pass-1 PSUM banks) ----
            with tc.tile_pool(name="ps2", bufs=1, space="PSUM") as ps2:
                if True:
                    g_sb = gs_sb[:, 0:128]
                    s_col = gs_sb[:, 128:129]

                    # s^T and (Wk s)^T as 1-partition rows
                    rows_ps = ps2.tile([1, 512], f32, tag="sm")
                    nc.tensor.matmul(
                        rows_ps[:, 0:128], lhsT=s_col, rhs=id_sb,
                        start=True, stop=True,
                    )
                    nc.tensor.matmul(
                        rows_ps[:, 128:256], lhsT=s_col, rhs=wkT_sb,
                        start=True, stop=True, skip_group_check=True,
                    )
                    rows_sb = sbres.tile([1, 256], f16)
                    nc.vector.tensor_copy(out=rows_sb[:], in_=rows_ps[:, 0:256])
                    srow_sb = rows_sb[:, 0:128]
                    kkrow_sb = rows_sb[:, 128:256]

                    # r2 = L*bk + (Wk s)^T   [1,128]
                    r2_sb = sbres.tile([1, 128], f16)
                    nc.vector.scalar_tensor_tensor(
                        out=r2_sb[:],
                        in0=bk_sb,
                        scalar=float(L),
                        in1=kkrow_sb,
                        op0=mybir.AluOpType.mult,
                        op1=mybir.AluOpType.add,
                    )

                    # V1 = G Wk^T + s bk^T
                    v1_ps = ps2.tile([128, 128], f32, tag="mm")
                    nc.tensor.matmul(
                        v1_ps[:], lhsT=g_sb, rhs=wkT_sb, start=True, stop=False
                    )
                    nc.tensor.matmul(
                        v1_ps[:], lhsT=srow_sb, rhs=bk_sb, start=False, stop=True
                    )
                    v1_sb = sbres.tile([128, 128], f16)
                    nc.vector.tensor_copy(out=v1_sb[:], in_=v1_ps[:])

                    # logits = Wq V1 + bq (outer) r2
                    lg_ps = ps2.tile([128, 128], f32, tag="mm2")
                    nc.tensor.matmul(
                        lg_ps[:], lhsT=wqT_sb, rhs=v1_sb[:], start=True, stop=False
                    )
                    nc.tensor.matmul(
                        lg_ps[:], lhsT=bq_sb, rhs=r2_sb[:], start=False, stop=True
                    )

                    # softmax over the free axis (ACT only does the exp)
                    negmax = sbres.tile([128, 1], f32)
                    nc.vector.tensor_reduce(
                        out=negmax[:], in_=lg_ps[:], axis=mybir.AxisListType.X,
                        op=mybir.AluOpType.max, negate=True,
                    )
                    a_sb = sbres.tile([128, 128], f16)
                    sumexp = sbres.tile([128, 1], f32)
                    nc.scalar.activation(
                        out=a_sb[:], in_=lg_ps[:],
                        func=mybir.ActivationFunctionType.Exp,
                        bias=negmax[:], scale=1.0, accum_out=sumexp[:],
                    )
                    rec = sbres.tile([128, 1], f32)
                    nc.vector.reciprocal(out=rec[:], in_=sumexp[:])
                    nc.vector.tensor_scalar_mul(a_sb[:], a_sb[:], rec[:])

                    # U = A^T Wo^T  [b, o]
                    u_ps = ps2.tile([128, 128], f32, tag="mm")
                    nc.tensor.matmul(
                        u_ps[:], lhsT=a_sb[:], rhs=woT_sb, start=True, stop=True
                    )
                    u_sb = sbres.tile([128, 128], f16)
                    nc.vector.tensor_copy(out=u_sb[:], in_=u_ps[:])

                    # M^T = Wv^T A^T Wo^T ; P^T = M^T + I
                    mt_ps = ps2.tile([128, 128], f32, tag="mm2")
                    nc.tensor.matmul(
                        mt_ps[:], lhsT=wv_sb, rhs=u_sb[:], start=True, stop=True
                    )
                    pt_sb = sbres.tile([128, 128], f16)
                    nc.vector.tensor_add(out=pt_sb[:], in0=mt_ps[:], in1=id_sb)

                    # cvec = U^T bv + bo  [o,1]
                    cv_ps = ps2.tile([128, 64], f32, tag="sm")
                    nc.tensor.matmul(
                        cv_ps[:, 0:1], lhsT=u_sb[:], rhs=bv_sb,
                        start=True, stop=True, skip_group_check=True,
                    )
                    cvec_sb = sbres.tile([128, 1], f32)
                    nc.vector.scalar_tensor_tensor(
                        out=cvec_sb[:],
                        in0=cv_ps[:, 0:1],
                        scalar=1.0,
                        in1=bo_sb,
                        op0=mybir.AluOpType.mult,
                        op1=mybir.AluOpType.add,
                    )

                    # out = (M + I) x_mean + cvec, 7 chunks of 448 columns;
                    # bias-adds alternate DVE/ACT, DMA per chunk on both queues
                    out_sb = sbres.tile([128, HW], f32)
                    for k in range(7):
                        oc_ps = ps2.tile([128, 448], f32, name=f"oc{k}", tag="oc", bufs=3)
                        nc.tensor.matmul(
                            oc_ps[:],
                            lhsT=pt_sb[:],
                            rhs=xm_sb[:, 448 * k : 448 * (k + 1)],
                            start=True, stop=True,
                        )
                        ob = out_sb[:, 448 * k : 448 * (k + 1)]
                        if k % 2 == 0:
                            nc.vector.tensor_scalar_add(ob, oc_ps[:], cvec_sb[:])
                        else:
                            nc.scalar.activation(
                                out=ob, in_=oc_ps[:],
                                func=mybir.ActivationFunctionType.Identity,
                                bias=cvec_sb[:], scale=1.0,
                            )
                        eng = nc.sync if k % 2 == 0 else nc.scalar
                        eng.dma_start(
                            out=out_d[:, 448 * k : 448 * (k + 1)], in_=ob
                        )

    _split_multi_waits(nc)
    _hoist_first_dmas(nc)
    return nc


_cached_nc = None


def kernel(x, w_q, b_q, w_k, b_k, w_v, b_v, w_o, b_o):
    global _cached_nc, _last_results
    from concourse.bass_utils import run_bass_kernel_spmd

    if _cached_nc is None:
        _cached_nc = _build_nc()
    nc = _cached_nc

    x = np.asarray(x, np.float32)
    pack = np.zeros((128, _PACKW), np.float16)
    pack[:, _WQ : _WQ + 128] = np.asarray(w_q, np.float32).T.astype(np.float16)
    pack[:, _WK : _WK + 128] = np.asarray(w_k, np.float32).T.astype(np.float16)
    pack[:, _WV : _WV + 128] = np.asarray(w_v, np.float32).astype(np.float16)
    pack[:, _WO : _WO + 128] = np.asarray(w_o, np.float32).T.astype(np.float16)
    pack[:, _ID : _ID + 128] = np.eye(128, dtype=np.float16)
    pack[:, _BV] = np.asarray(b_v, np.float16)
    pack[:, _BO] = np.asarray(b_o, np.float16)
    pack[0, _BQ : _BQ + 128] = np.asarray(b_q, np.float16)
    pack[0, _BK : _BK + 128] = np.asarray(b_k, np.float16)
    ic16 = np.ascontiguousarray((np.eye(128) / 16.0).astype(np.float16))

    # position 2k holds chunk k, position 2k+1 holds chunk k+196 (these two
    # share an x_mean window, letting the device fold them with one fp16 add)
    order = np.empty(T, np.int64)
    order[0::2] = np.arange(TF)
    order[1::2] = np.arange(TF) + TF
    in_maps = []
    for b in range(B):
        # xt[p, CW*t + c] = x[b, c, 128*t + p] for c < 128; ones at c == 128
        xb = x[b].reshape(C, T, 128)
        xt_b = np.zeros((128, T, CW), np.float16)
        xt_b[:, :, :128] = xb.transpose(2, 1, 0)[:, order, :].astype(np.float16)
        xt_b[:, :, 128] = np.float16(1.0)
        in_maps.append(
            {"xt": xt_b.reshape(128, T * CW), "ic16": ic16, "pack": pack}
        )

    res = run_bass_kernel_spmd(nc, in_maps, list(range(N_CORES)))
    _last_results = res

    out = np.empty((B, C, H, W), np.float32)
    for b in range(B):
        out[b] = res.results[b]["out"].reshape(C, H, W)
    return out
